# revision 1
# baseline (speedup 1.0000x reference)
"""Trainium2 Bass kernel for a GQA sliding-window attention layer.

Reference computation (B=2, T=2048, C=2048, 16 Q heads / 4 KV heads, d=128):
    q = x @ Wq; k = x @ Wk; v = x @ Wv (+ sigmoid-gated value embedding)
    q, k = rmsnorm(rope(q)), rmsnorm(rope(k))
    scores masked to the band 0 <= j - i < window (=1024), softmax over j
    out = (p @ v) @ Wo

Sharding: 8 cores = 2 batches x 4 KV groups.  Each core computes its 4 Q
heads / 1 KV head for one batch and a partial output (its 512-row slice of
the Wo contraction); the host sums the 4 partials per batch.

Layout strategy per core:
  - xT (C x T, bf16) resident in SBUF; all projections contract over C.
  - q̂T / k̂T kept [d=128 partitions, T free]; scores computed transposed
    (S^T tiles [kj, qi]) so that P^T feeds the PV matmul directly with v in
    natural [token, d] layout (no P transposes).
  - softmax has no max-subtraction: rms-normalized q,k bound |score| by
    sqrt(128), so exp is safe in fp32.
  - per-q softmax denominators and rms rows are broadcast across partitions
    via a tiny DRAM bounce (SBUF APs need nonzero partition stride).
"""

import numpy as np
import ml_dtypes
from collections import deque

BF16 = ml_dtypes.bfloat16

# Problem dims (hardcoded per contest rules)
B, T, C = 2, 2048, 2048
N_HEAD, N_KV, HD, GATE_CH = 16, 4, 128, 32
WINDOW = 1024
P = 128
GH = N_HEAD // N_KV  # q heads per kv head (= per core)
N_CORES = 8

_PROGRAM_CACHE = {}


def build_program(T_=T, C_=C, win=WINDOW):
    import concourse.mybir as mybir
    import concourse.tile as tile
    from concourse import bacc

    dt = mybir.dt
    f32 = dt.float32
    bf16 = dt.bfloat16
    AF = mybir.ActivationFunctionType
    ALU = mybir.AluOpType

    NT = T_ // P          # token tiles
    KT = C_ // P          # contraction tiles
    WT = win // P         # window tiles
    ISQ = 1.0 / float(np.sqrt(HD))

    nc = bacc.Bacc()

    xT = nc.declare_dram_parameter("xT", [C_, T_], bf16, isOutput=False)
    wq = nc.declare_dram_parameter("wq", [C_, GH * HD], bf16, isOutput=False)
    wk = nc.declare_dram_parameter("wk", [C_, HD], bf16, isOutput=False)
    wv = nc.declare_dram_parameter("wv", [C_, HD], bf16, isOutput=False)
    wg = nc.declare_dram_parameter("wg", [GATE_CH, 1], bf16, isOutput=False)
    ve2 = nc.declare_dram_parameter("ve2", [T_, HD], bf16, isOutput=False)
    wo = nc.declare_dram_parameter("wo", [GH * HD, C_], bf16, isOutput=False)
    ccd = nc.declare_dram_parameter("cc", [P, T_], bf16, isOutput=False)
    ssd = nc.declare_dram_parameter("ss", [P, T_], bf16, isOutput=False)
    tlo = nc.declare_dram_parameter("tlo", [P, P], bf16, isOutput=False)
    thi = nc.declare_dram_parameter("thi", [P, P], bf16, isOutput=False)
    idr = nc.declare_dram_parameter("identr", [P, GH * P], bf16, isOutput=False)
    idf = nc.declare_dram_parameter("identf", [P, P], f32, isOutput=False)
    out_d = nc.declare_dram_parameter("out", [T_, C_], f32, isOutput=True)
    f32r = dt.float32r

    with tile.TileContext(nc) as tc:
        with (
            tc.tile_pool(name="singles", bufs=1) as sg,
            tc.tile_pool(name="work", bufs=2) as wk_pool,
            tc.tile_pool(name="work3", bufs=4) as w3_pool,
            tc.tile_pool(name="attw", bufs=4) as aw,
            tc.tile_pool(name="yup", bufs=3) as yu_pool,
            tc.tile_pool(name="outp", bufs=3) as op_pool,
            tc.tile_pool(name="psum", bufs=8, space="PSUM") as pp,
        ):
            # ---- persistent inputs -------------------------------------
            # weight/x DMAs are split per k-tile and interleaved so the
            # first projection matmuls (kt=0) can start almost immediately
            # small constants FIRST (the rope tails read cc/ss early — they
            # must not queue behind the 13MB of x/weight traffic)
            wg_sb = sg.tile([GATE_CH, 1], bf16, tag="wg")
            nc.sync.dma_start(out=wg_sb[:], in_=wg[:])
            cc_sb = sg.tile([P, T_], bf16, tag="cc")
            nc.sync.dma_start(out=cc_sb[:], in_=ccd[:])
            ss_sb = sg.tile([P, T_], bf16, tag="ss")
            nc.sync.dma_start(out=ss_sb[:], in_=ssd[:])
            ve2_sb = sg.tile([P, NT, HD], bf16, tag="ve2")
            nc.sync.dma_start(out=ve2_sb[:], in_=ve2.rearrange("(o p) d -> p o d", p=P))
            tlo_sb = sg.tile([P, P], bf16, tag="tlo")
            nc.sync.dma_start(out=tlo_sb[:], in_=tlo[:])
            thi_sb = sg.tile([P, P], bf16, tag="thi")
            nc.sync.dma_start(out=thi_sb[:], in_=thi[:])
            idr_sb = sg.tile([P, GH * P], bf16, tag="idr")
            nc.sync.dma_start(out=idr_sb[:], in_=idr[:])
            idf_sb = sg.tile([P, P], f32, tag="idf")
            nc.sync.dma_start(out=idf_sb[:], in_=idf[:])
            xt = []
            wq_sb = sg.tile([P, KT, GH * HD], bf16, tag="wq")
            wk_sb = sg.tile([P, KT, HD], bf16, tag="wk")
            wv_sb = sg.tile([P, KT, HD], bf16, tag="wv")
            wqr = wq.rearrange("(o p) n -> p o n", p=P)
            wkr = wk.rearrange("(o p) n -> p o n", p=P)
            wvr = wv.rearrange("(o p) n -> p o n", p=P)
            for kt in range(KT):
                t_ = sg.tile([P, T_], bf16, tag=f"xt{kt}")
                nc.sync.dma_start(out=wk_sb[:, kt, :], in_=wkr[:, kt, :])
                nc.sync.dma_start(out=t_[:], in_=xT[kt * P:(kt + 1) * P, :])
                xt.append(t_)
                nc.sync.dma_start(out=wq_sb[:, kt, :], in_=wqr[:, kt, :])
                nc.sync.dma_start(out=wv_sb[:, kt, :], in_=wvr[:, kt, :])
            wo_sb = sg.tile([P, GH, C_], bf16, tag="wo")
            nc.sync.dma_start(out=wo_sb[:], in_=wo.rearrange("(o p) n -> p o n", p=P))
            ones_sb = sg.tile([P, 1], bf16, tag="onesb")
            nc.vector.memset(ones_sb[:], 1.0)
            ones1f = sg.tile([1, P], f32, tag="ones1f")
            nc.vector.memset(ones1f[:], 1.0)
            eps_sb = sg.tile([P, 1], f32, tag="epsb")
            nc.vector.memset(eps_sb[:], 1e-6)

            # persistent intermediates
            qhat = sg.tile([P, GH, T_], bf16, tag="qhat")   # normalized roped q, [d, h, t]
            khat = sg.tile([P, T_], bf16, tag="khat")       # normalized roped k * isq
            vsb = sg.tile([P, NT, HD], bf16, tag="vsb")     # gated v, [tok, tt, d]

            TS = T_ // 512  # 512-wide token slices

            # ---- projections + rope + rmsnorm for k/q heads and vT -----
            # Emitted as kt-major WAVES of 3 output groups: the PE chases the
            # xT DMAs tile-by-tile during the ramp, and each wave's dependent
            # tail work (rope/rms/broadcast) is batched behind the next
            # wave's matmuls so the PE stream never waits on DVE/ACT chains.
            def wave_mms(wave):
                items = []
                for (head, ts_) in wave:
                    sl = slice(ts_ * 512, ts_ * 512 + 512)
                    ps = pp.tile([P, 512], f32, tag="pb",
                                 name=f"ps{head}_{ts_}")
                    items.append((head, sl, ps))
                for kt in range(KT):
                    for gi, (head, ts_) in enumerate(wave):
                        if head == 0:
                            w_ap = wk_sb[:, kt, :]
                        elif head == GH + 1:
                            w_ap = wv_sb[:, kt, :]
                        else:
                            w_ap = wq_sb[:, kt, (head - 1) * HD:head * HD]
                        nc.tensor.matmul(
                            items[gi][2][:], lhsT=w_ap,
                            rhs=xt[kt][:, items[gi][1]],
                            start=(kt == 0), stop=(kt == KT - 1),
                        )
                return items

            def v_tail(head, sl, ps):
                # vT psum [d, tok] -> sbuf f32, then PE-transpose each 128-tok
                # block to natural [tok, d] and add the sigmoid-gated ve.
                vt = wk_pool.tile([P, 512], f32, tag="vt")
                nc.vector.tensor_copy(vt[:], ps[:])
                for i in range(4):
                    tt = sl.start // P + i
                    tsl = slice(tt * P, (tt + 1) * P)
                    tp = pp.tile([P, P], f32, tag="pb")
                    nc.tensor.transpose(tp[:], vt[:, i * P:(i + 1) * P], idf_sb[:])
                    gps = pp.tile([P, 1], f32, tag="pb")
                    nc.tensor.matmul(gps[:], lhsT=xt[0][0:GATE_CH, tsl],
                                     rhs=wg_sb[:], start=True, stop=True)
                    gcol = wk_pool.tile([P, 1], f32, tag="gcol")
                    nc.scalar.activation(gcol[:], gps[:], AF.Sigmoid)
                    # v = ve2 * sigmoid(g) + v_proj (ve2 pre-scaled by 2)
                    nc.vector.scalar_tensor_tensor(
                        out=vsb[:, tt, :], in0=ve2_sb[:, tt, :], scalar=gcol[:],
                        in1=tp[:], op0=ALU.mult, op1=ALU.add,
                    )

            def wave_tails(items):
                t1 = []
                for (head, sl, ps) in items:
                    if head == GH + 1:
                        v_tail(head, sl, ps)
                        continue
                    # rope: qr = ps*cc + swap(ps)*ss  (ss carries the sign)
                    qr = w3_pool.tile([P, 512], f32, tag="qr")
                    nc.vector.tensor_mul(qr[:], ps[:], cc_sb[:, sl])
                    qs = wk_pool.tile([P, 512], f32, tag="qs")
                    nc.vector.tensor_mul(qs[0:64, :], ps[64:128, :],
                                         ss_sb[0:64, sl])
                    nc.vector.tensor_mul(qs[64:128, :], ps[0:64, :],
                                         ss_sb[64:128, sl])
                    nc.vector.tensor_add(qr[:], qr[:], qs[:])
                    q2 = wk_pool.tile([P, 512], bf16, tag="q2")
                    nc.gpsimd.tensor_mul(q2[:], qr[:], qr[:])
                    t1.append((head, sl, qr, q2))
                ssqs = []
                for (head, sl, qr, q2) in t1:
                    ssq = pp.tile([1, 512], f32, tag="pb")
                    nc.tensor.matmul(ssq[:], lhsT=ones_sb[:], rhs=q2[:],
                                     start=True, stop=True)
                    ssqs.append(ssq)
                rows = []
                for (head, sl, qr, q2), ssq in zip(t1, ssqs):
                    srow = w3_pool.tile([1, 512], f32, tag="srow")
                    nc.scalar.activation(srow[:], ssq[:], AF.Sqrt,
                                         bias=eps_sb[0:1, :], scale=1.0 / HD)
                    rows.append(srow)
                rrs = []
                for (head, sl, qr, q2), srow in zip(t1, rows):
                    rr = w3_pool.tile([1, 512], f32, tag="rr")
                    nc.vector.reciprocal_approx_fast(rr[:], srow[:])
                    if head == 0:
                        # fold the 1/sqrt(d) score scale into k-hat
                        nc.vector.tensor_scalar_mul(rr[:], rr[:], ISQ)
                    rrs.append(rr)
                rrbs = []
                for (head, sl, qr, q2), rr in zip(t1, rrs):
                    rrb = pp.tile([P, 512], f32, tag="pb")
                    nc.tensor.matmul(rrb[:], lhsT=ones1f[:], rhs=rr[:],
                                     start=True, stop=True)
                    rrbs.append(rrb)
                for (head, sl, qr, q2), rrb in zip(t1, rrbs):
                    dest = khat[:, sl] if head == 0 else qhat[:, head - 1, sl]
                    nc.vector.tensor_mul(dest, qr[:], rrb[:])

            groups = [(head, ts_) for head in range(GH + 2)
                      for ts_ in range(TS)]
            prev_items = None
            for w0 in range(0, len(groups), 3):
                items = wave_mms(groups[w0:w0 + 3])
                if prev_items:
                    wave_tails(prev_items)
                prev_items = items
            wave_tails(prev_items)

            CO = C_ // 512  # output column chunks
            # All 4 q-heads are fused into one 512-wide moving operand:
            # scores / exp / den / PV are each ONE N=512 instruction per
            # (qi, kt), so LDWEIGHTS fully hides under the matmul stream.
            denps = {}
            yps = {}
            yus = {}
            rds = {}

            def attn_scores_k(qi, kk):
                ktc = min(WT + 1, NT - qi)
                qs4 = qhat[:, :, qi * P:(qi + 1) * P]   # [d, (h, q)] = 512 wide
                kt = qi + kk
                sp = pp.tile([P, GH * P], f32, tag="pb")
                masked = (kk == 0) or (kk == WT and ktc == WT + 1)
                nc.tensor.matmul(
                    sp[:], lhsT=khat[:, kt * P:(kt + 1) * P], rhs=qs4,
                    start=True, stop=not masked,
                )
                if masked:
                    # band-mask bias (-3e4 outside band): psum += bias.T @ I_rep
                    nc.tensor.matmul(
                        sp[:], lhsT=tlo_sb[:] if kk == 0 else thi_sb[:],
                        rhs=idr_sb[:], start=False, stop=True,
                    )
                pt = aw.tile([P, GH * P], bf16, tag="pT")
                nc.scalar.activation(pt[:], sp[:], AF.Exp)
                return pt

            def attn_pv_k(qi, kk, pt):
                ktc = min(WT + 1, NT - qi)
                if kk == 0:
                    denps[qi] = pp.tile([1, GH * P], f32, tag="pb",
                                        name=f"denp{qi}")
                    yps[qi] = pp.tile([P, GH * P], f32, tag="pb",
                                      name=f"yp{qi}")
                kt = qi + kk
                nc.tensor.matmul(
                    denps[qi][:], lhsT=ones_sb[:], rhs=pt[:],
                    start=(kk == 0), stop=(kk == ktc - 1),
                )
                nc.tensor.matmul(
                    yps[qi][:], lhsT=vsb[:, kt, :], rhs=pt[:],
                    start=(kk == 0), stop=(kk == ktc - 1),
                )
                if kk == ktc - 1:
                    yut = yu_pool.tile([P, GH * P], f32, tag="yu")
                    nc.vector.tensor_copy(yut[:], yps[qi][:])
                    yus[qi] = yut
                    rd = wk_pool.tile([1, GH * P], f32, tag="rd")
                    nc.vector.reciprocal_approx_fast(rd[:], denps[qi][:])
                    rds[qi] = rd

            def attn_out(qi):
                qsl = slice(qi * P, (qi + 1) * P)
                rdb = pp.tile([P, GH * P], f32, tag="pb")
                nc.tensor.matmul(rdb[:], lhsT=ones1f[:], rhs=rds[qi][:],
                                 start=True, stop=True)
                yq = op_pool.tile([P, GH * P], bf16, tag="yq")
                nc.vector.tensor_mul(yq[:], yus[qi][:], rdb[:])
                for co in range(CO):
                    osl = slice(co * 512, co * 512 + 512)
                    ops = pp.tile([P, 512], f32, tag="pb")
                    for h in range(GH):
                        nc.tensor.matmul(
                            ops[:], lhsT=yq[:, h * P:(h + 1) * P],
                            rhs=wo_sb[:, h, osl],
                            start=(h == 0), stop=(h == GH - 1),
                        )
                    ob = op_pool.tile([P, 512], f32, tag="ob")
                    nc.vector.tensor_copy(out=ob[:], in_=ops[:])
                    nc.sync.dma_start(out=out_d[qsl, osl], in_=ob[:])

            pv_queue = deque()
            done_out = set()
            out_ready = deque()
            for qi in range(NT):
                ktc = min(WT + 1, NT - qi)
                for kk in range(ktc):
                    pt = attn_scores_k(qi, kk)
                    if len(pv_queue) >= 2:
                        attn_pv_k(*pv_queue.popleft())
                    pv_queue.append((qi, kk, pt))
                    # emit out-proj one iteration after its recip is queued,
                    # so the PE never waits on the denominator chain
                    if out_ready and out_ready[0][1] <= 0:
                        done_out.add(out_ready[0][0])
                        attn_out(out_ready.popleft()[0])
                    out_ready = deque([(q, age - 1) for q, age in out_ready])
                    if qi > 0 and (qi - 1) in rds and (qi - 1) not in done_out \
                            and all(q != qi - 1 for q, _ in out_ready):
                        out_ready.append((qi - 1, 1))
            while pv_queue:
                attn_pv_k(*pv_queue.popleft())
            for qi in range(NT):
                if qi not in done_out:
                    attn_out(qi)

    return nc


def _get_program(T_=T, C_=C, win=WINDOW):
    key = (T_, C_, win)
    if key not in _PROGRAM_CACHE:
        nc = build_program(T_, C_, win)
        nc.finalize()
        _PROGRAM_CACHE[key] = nc
    return _PROGRAM_CACHE[key]


def make_in_maps(x, ve, cos, sin, Wq, Wk, Wv, Wg, Wo):
    """Build the 8 per-core input dicts (host-side sharding/layout prep)."""
    cosT = np.ascontiguousarray(cos[:, 0, :].T).astype(np.float32)  # [64, T]
    sinT = np.ascontiguousarray(sin[:, 0, :].T).astype(np.float32)
    cc = np.concatenate([cosT, cosT], axis=0)            # [128, T]
    ss = np.concatenate([sinT, -sinT], axis=0)           # [128, T]
    # additive mask biases for the S^T diagonal/far tiles, pre-transposed
    # (they enter the scores as lhsT with an identity rhs: psum += bias.T)
    neg = np.float32(-30000.0)
    bias_lo = np.where(np.arange(P)[:, None] >= np.arange(P)[None, :], 0.0, neg)
    bias_hi = np.where(np.arange(P)[:, None] < np.arange(P)[None, :], 0.0, neg)
    tlo = np.ascontiguousarray(bias_lo.T).astype(BF16)
    thi = np.ascontiguousarray(bias_hi.T).astype(BF16)
    identr = np.tile(np.eye(P, dtype=np.float32), (1, GH)).astype(BF16)
    identf = np.eye(P, dtype=np.float32)

    in_maps = []
    for core in range(N_CORES):
        b, g = divmod(core, N_KV)
        in_maps.append({
            "xT": np.ascontiguousarray(x[b].T).astype(BF16),
            "wq": Wq[:, g * GH * HD:(g + 1) * GH * HD].astype(BF16),
            "wk": Wk[:, g * HD:(g + 1) * HD].astype(BF16),
            "wv": Wv[:, g * HD:(g + 1) * HD].astype(BF16),
            "wg": np.ascontiguousarray(Wg[:, g:g + 1]).astype(BF16),
            "ve2": (2.0 * ve[b][:, g * HD:(g + 1) * HD]).astype(BF16),
            "wo": Wo[g * GH * HD:(g + 1) * GH * HD, :].astype(BF16),
            "cc": cc.astype(BF16), "ss": ss.astype(BF16),
            "tlo": tlo, "thi": thi, "identr": identr, "identf": identf,
        })
    return in_maps


def kernel(x, ve, cos, sin, Wq, Wk, Wv, Wg, Wo, window):
    assert int(window) == WINDOW and x.shape == (B, T, C)
    from concourse.bass_utils import run_bass_kernel_spmd

    nc = _get_program()
    in_maps = make_in_maps(x, ve, cos, sin, Wq, Wk, Wv, Wg, Wo)
    res = run_bass_kernel_spmd(nc, in_maps, core_ids=list(range(N_CORES)))
    out = np.zeros((B, T, C), dtype=np.float32)
    for core in range(N_CORES):
        b = core // N_KV
        out[b] += res.results[core]["out"]
    return out



# revision 16
# speedup vs baseline: 1.0625x; 1.0625x over previous
"""Trainium2 Bass kernel for a GQA sliding-window attention layer.

Reference computation (B=2, T=2048, C=2048, 16 Q heads / 4 KV heads, d=128):
    q = x @ Wq; k = x @ Wk; v = x @ Wv (+ sigmoid-gated value embedding)
    q, k = rmsnorm(rope(q)), rmsnorm(rope(k))
    scores masked to the band 0 <= j - i < window (=1024), softmax over j
    out = (p @ v) @ Wo

Sharding: 8 cores = 2 batches x 4 KV groups.  Each core computes its 4 Q
heads / 1 KV head for one batch and a partial output (its 512-row slice of
the Wo contraction); the host sums the 4 partials per batch.

Key implementation notes:
  - fp16 everywhere (same PE/DVE speed as bf16, 8x the mantissa bits).
  - every PE matmul moves fp16 at 1 cycle/row; the only non-GEMM PE work
    is tiny [1,512] row-sums (rms ssq, softmax den) and their fp16
    [128,512] broadcast matmuls (213ns each at full clock).
  - band masks are 0/1 fp16 multiplies on the DVE (4x mode via
    scalar_tensor_tensor) applied to exp(scores) tiles.
  - softmax denominator: fp16 DVE accumulation of the exp tiles, then one
    row-sum matmul per 128-query row.
  - rope reads the projection PSUM directly (the half-swap addressing is
    only legal with a PSUM operand); the swapped-half multiplies run on
    the otherwise-idle Pool engine.
  - gate sigmoid is computed via Exp (1/(1+e^-x)) at the start of the
    attention phase so the ACT engine needs only one table set per phase
    (Sqrt during projection, Exp afterwards).
  - exp has a -2.0 bias folded in (cancels in the softmax ratio) so fp16
    can't overflow even for adversarially aligned q/k.
  - x is DMA'd in token-slice-major order so the first projection wave
    starts after ~1/16 of x has arrived; out is written as fp16.
"""

import numpy as np
from collections import deque

F16 = np.float16

# Problem dims (hardcoded per contest rules)
B, T, C = 2, 2048, 2048
N_HEAD, N_KV, HD, GATE_CH = 16, 4, 128, 32
WINDOW = 1024
P = 128
GH = N_HEAD // N_KV  # q heads per kv head (= per core)
N_CORES = 8

_PROGRAM_CACHE = {}


def build_program(T_=T, C_=C, win=WINDOW, debug=False):
    import concourse.mybir as mybir
    import concourse.tile as tile
    from concourse import bacc

    dt = mybir.dt
    f32 = dt.float32
    f16 = dt.float16
    AF = mybir.ActivationFunctionType
    ALU = mybir.AluOpType

    NT = T_ // P          # token tiles
    KT = C_ // P          # contraction tiles
    WT = win // P         # window tiles
    TS = T_ // 512        # 512-token slices

    nc = bacc.Bacc()

    xT = nc.declare_dram_parameter("xT", [C_, T_], f16, isOutput=False)
    wq = nc.declare_dram_parameter("wq", [C_, GH * HD], f16, isOutput=False)
    wk = nc.declare_dram_parameter("wk", [C_, HD], f16, isOutput=False)
    wv = nc.declare_dram_parameter("wv", [C_, HD], f16, isOutput=False)
    wg = nc.declare_dram_parameter("wg", [GATE_CH, 1], f16, isOutput=False)
    ve2 = nc.declare_dram_parameter("ve2", [T_, HD], f16, isOutput=False)
    wo = nc.declare_dram_parameter("wo", [GH * HD, C_], f16, isOutput=False)
    ccd = nc.declare_dram_parameter("cc", [P, T_], f16, isOutput=False)
    ssd = nc.declare_dram_parameter("ss", [P, T_], f16, isOutput=False)
    mlod = nc.declare_dram_parameter("mlo", [P, GH * P], f16, isOutput=False)
    mhid = nc.declare_dram_parameter("mhi", [P, GH * P], f16, isOutput=False)
    out_d = nc.declare_dram_parameter("out", [T_, C_], f16, isOutput=True)
    if debug:
        NTd = T_ // P
        khat_d = nc.declare_dram_parameter("khat_d", [P, T_], f16, isOutput=True)
        qhat_d = nc.declare_dram_parameter("qhat_d", [P, N_HEAD // N_KV, T_], f16, isOutput=True)
        vsb_d = nc.declare_dram_parameter("vsb_d", [P, NTd, HD], f16, isOutput=True)
        sig_d = nc.declare_dram_parameter("sig_d", [P, NTd], f32, isOutput=True)
        pacc_d = nc.declare_dram_parameter("pacc_d", [P, NTd, 512], f16, isOutput=True)

    with tile.TileContext(nc) as tc:
        with (
            tc.tile_pool(name="singles", bufs=1) as sg,
            tc.tile_pool(name="tails", bufs=3) as tl,
            tc.tile_pool(name="attn", bufs=4) as aw,
            tc.tile_pool(name="outp", bufs=3) as op_pool,
            tc.tile_pool(name="psum", bufs=1, space="PSUM") as pp,
        ):
            # ---- input DMAs --------------------------------------------
            wg_sb = sg.tile([GATE_CH, 1], f16, tag="wg")
            nc.sync.dma_start(out=wg_sb[:], in_=wg[:])
            wq_sb = sg.tile([P, KT, GH * HD], f16, tag="wq")
            wk_sb = sg.tile([P, KT, HD], f16, tag="wk")
            wv_sb = sg.tile([P, KT, HD], f16, tag="wv")
            wqr = wq.rearrange("(o p) n -> p o n", p=P)
            wkr = wk.rearrange("(o p) n -> p o n", p=P)
            wvr = wv.rearrange("(o p) n -> p o n", p=P)
            xt = []
            for kt in range(KT):
                t_ = sg.tile([P, T_], f16, tag=f"xt{kt}", name=f"xt{kt}")
                xt.append(t_)
            sl0 = slice(0, 512)
            for kt in range(KT):
                nc.sync.dma_start(out=wk_sb[:, kt, :], in_=wkr[:, kt, :])
                nc.sync.dma_start(out=xt[kt][:, sl0],
                                  in_=xT[kt * P:(kt + 1) * P, sl0])
                nc.sync.dma_start(out=wq_sb[:, kt, :], in_=wqr[:, kt, :])
                nc.sync.dma_start(out=wv_sb[:, kt, :], in_=wvr[:, kt, :])
            # constants needed by the first wave's tails
            cc_sb = sg.tile([P, T_], f16, tag="cc")
            nc.sync.dma_start(out=cc_sb[:], in_=ccd[:])
            ss_sb = sg.tile([P, T_], f16, tag="ss")
            nc.sync.dma_start(out=ss_sb[:], in_=ssd[:])
            ve2_sb = sg.tile([P, NT, HD], f16, tag="ve2")
            nc.sync.dma_start(out=ve2_sb[:], in_=ve2.rearrange("(o p) d -> p o d", p=P))
            mlo_sb = sg.tile([P, GH * P], f16, tag="mlo")
            nc.sync.dma_start(out=mlo_sb[:], in_=mlod[:])
            mhi_sb = sg.tile([P, GH * P], f16, tag="mhi")
            nc.sync.dma_start(out=mhi_sb[:], in_=mhid[:])
            for ts in range(1, TS):
                sl = slice(ts * 512, (ts + 1) * 512)
                for kt in range(KT):
                    nc.sync.dma_start(out=xt[kt][:, sl],
                                      in_=xT[kt * P:(kt + 1) * P, sl])
            wo_sb = sg.tile([P, GH, C_], f16, tag="wo")
            nc.sync.dma_start(out=wo_sb[:], in_=wo.rearrange("(o p) n -> p o n", p=P))

            ones_sb = sg.tile([P, 1], f16, tag="onesb")
            nc.vector.memset(ones_sb[:], 1.0)
            ones1h = sg.tile([1, P], f16, tag="ones1h")
            nc.vector.memset(ones1h[:], 1.0)
            eps_sb = sg.tile([P, 1], f32, tag="epsb")
            nc.vector.memset(eps_sb[:], 1e-6)
            # k gets the 1/sqrt(d) score scale folded into its rms scale:
            # rr_k = (1/sqrt(ssq/HD+eps))/sqrt(HD) = 1/sqrt(ssq + HD*eps)
            epsk_sb = sg.tile([P, 1], f32, tag="epskb")
            nc.vector.memset(epsk_sb[:], HD * 1e-6)
            nexp_b = sg.tile([P, 1], f32, tag="nexpb")
            nc.vector.memset(nexp_b[:], -2.0)

            # persistent intermediates
            qhat = sg.tile([P, GH, T_], f16, tag="qhat")   # normalized roped q, [d, h, t]
            khat = sg.tile([P, T_], f16, tag="khat")       # normalized roped k * isq
            vsb = sg.tile([P, NT, HD], f16, tag="vsb")     # v (gated during attn start)
            sig_sb = sg.tile([P, NT], f32, tag="sigsb")    # gate sigmoids

            # ---- projection phase --------------------------------------
            gps = pp.tile([P, NT], f32, tag="gps", bufs=1)

            def wave_mms(ts, heads, tags):
                sl = slice(ts * 512, ts * 512 + 512)
                items = []
                for head, tag in zip(heads, tags):
                    nb = {"sp": 3, "yps": 2, "ops": 2}[tag]
                    if head == GH + 1:  # V group, [tok, d] per token tile
                        ps = pp.tile([P, 4, HD], f32, tag=tag, bufs=nb,
                                     name=f"psv{ts}")
                    else:
                        ps = pp.tile([P, 512], f32, tag=tag, bufs=nb,
                                     name=f"ps{head}_{ts}")
                    items.append((head, ts, ps))
                for kt in range(KT):
                    for head, _, ps in items:
                        if head == GH + 1:
                            continue
                        if head == 0:
                            nc.tensor.matmul(
                                ps[:], lhsT=wk_sb[:, kt, :],
                                rhs=xt[kt][:, sl],
                                start=(kt == 0), stop=(kt == KT - 1))
                        else:
                            nc.tensor.matmul(
                                ps[:], lhsT=wq_sb[:, kt, (head - 1) * HD:head * HD],
                                rhs=xt[kt][:, sl],
                                start=(kt == 0), stop=(kt == KT - 1))
                # the V chains share one psum bank, so a chain's start=True
                # would mark the whole bank pending-zero and clobber any
                # other open chain's first term: run them strictly one at a
                # time (j outer, kt inner).
                for head, _, ps in items:
                    if head != GH + 1:
                        continue
                    for j in range(4):
                        tsl = slice(ts * 512 + j * P, ts * 512 + (j + 1) * P)
                        for kt in range(KT):
                            nc.tensor.matmul(
                                ps[:, j, :], lhsT=xt[kt][:, tsl],
                                rhs=wv_sb[:, kt, :],
                                start=(kt == 0), stop=(kt == KT - 1),
                                skip_group_check=True)
                if heads[-1] == GH + 1:
                    for j in range(4):
                        tt = ts * 4 + j
                        tsl = slice(tt * P, (tt + 1) * P)
                        nc.tensor.matmul(
                            gps[:, tt:tt + 1], lhsT=xt[0][0:GATE_CH, tsl],
                            rhs=wg_sb[:], start=True, stop=True,
                            skip_group_check=True)
                return items

            def tails(items):
                # 1) ropes: free the projection psums first.  The
                #    half-swapped multiplies read the PSUM (partition cross
                #    is only legal there) and run on the Pool engine.
                t1 = []
                for head, ts, ps in items:
                    if head == GH + 1:
                        # raw v copy [tok, d] -> SBUF (gating happens later)
                        nc.scalar.copy(vsb[:, ts * 4:(ts + 1) * 4, :], ps[:])
                        continue
                    sl = slice(ts * 512, ts * 512 + 512)
                    qr = tl.tile([P, 512], f16, tag="qr", bufs=5)
                    nc.vector.tensor_mul(qr[:], ps[:], cc_sb[:, sl])
                    qs = tl.tile([P, 512], f16, tag="qs", bufs=5)
                    nc.vector.tensor_mul(qs[0:64, :], ps[64:128, :], ss_sb[0:64, sl])
                    nc.vector.tensor_mul(qs[64:128, :], ps[0:64, :], ss_sb[64:128, sl])
                    nc.vector.scalar_tensor_tensor(
                        out=qr[:], in0=qr[:], scalar=1.0, in1=qs[:],
                        op0=ALU.mult, op1=ALU.add)
                    t1.append((head, sl, qr))
                # 2) squares on Pool
                q2s = []
                for head, sl, qr in t1:
                    q2 = tl.tile([P, 512], f16, tag="q2", bufs=5)
                    nc.gpsimd.tensor_mul(q2[:], qr[:], qr[:])
                    q2s.append(q2)
                # 3) per group: row-sum matmul, sqrt, recip, cast, broadcast
                #    matmul, final scale.  sp-tag psums rotate: each is freed
                #    by the fast ACT/DVE op right behind it.
                for (head, sl, qr), q2 in zip(t1, q2s):
                    ssq = pp.tile([1, 512], f32, tag="sp", bufs=3, name="ssq")
                    nc.tensor.matmul(ssq[:], lhsT=ones_sb[:], rhs=q2[:],
                                     start=True, stop=True)
                    sq = tl.tile([1, 512], f32, tag="sq", bufs=4)
                    if head == 0:
                        nc.scalar.activation(sq[:], ssq[:], AF.Sqrt,
                                             bias=epsk_sb[0:1, :], scale=1.0)
                    else:
                        nc.scalar.activation(sq[:], ssq[:], AF.Sqrt,
                                             bias=eps_sb[0:1, :], scale=1.0 / HD)
                    rr = tl.tile([1, 512], f32, tag="rr", bufs=4)
                    nc.vector.reciprocal_approx_fast(rr[:], sq[:])
                    rr16 = tl.tile([1, 512], f16, tag="rr16", bufs=4)
                    nc.scalar.copy(rr16[:], rr[:])
                    rrb = pp.tile([P, 512], f32, tag="sp", bufs=3, name="rrb")
                    nc.tensor.matmul(rrb[:], lhsT=ones1h[:], rhs=rr16[:],
                                     start=True, stop=True)
                    dest = khat[:, sl] if head == 0 else qhat[:, head - 1, sl]
                    nc.vector.tensor_mul(dest, qr[:], rrb[:])

            for ts in range(TS):
                items_a = wave_mms(ts, [0, 1, 2], ["sp", "sp", "sp"])
                items_b = wave_mms(ts, [3, 4, GH + 1], ["yps", "yps", "ops"])
                tails(items_a)
                tails(items_b)

            # ---- gate sigmoids + v gating (one Exp table load) ---------
            egate = tl.tile([P, NT], f32, tag="egate", bufs=1)
            nc.scalar.activation(egate[:], gps[:], AF.Exp, scale=-1.0)  # e^-g
            ep1 = tl.tile([P, NT], f32, tag="ep1", bufs=1)
            nc.vector.tensor_scalar_add(ep1[:], egate[:], 1.0)
            nc.vector.reciprocal_approx_fast(sig_sb[:], ep1[:])         # sigmoid
            for tt in range(NT):
                # v += sigmoid * ve2 (ve2 pre-scaled by 2), in place
                nc.vector.scalar_tensor_tensor(
                    out=vsb[:, tt, :], in0=ve2_sb[:, tt, :],
                    scalar=sig_sb[:, tt:tt + 1],
                    in1=vsb[:, tt, :], op0=ALU.mult, op1=ALU.add)

            # ---- attention + out-projection ----------------------------
            CO = C_ // 512  # output column chunks

            yps_of = {}
            pacc_of = {}

            def attn_step(qi, kk):
                ktc = min(WT + 1, NT - qi)
                kt = qi + kk
                qs4 = qhat[:, :, qi * P:(qi + 1) * P]   # [d, (h, q)] = 512 wide
                sp = pp.tile([P, 512], f32, tag="sp", bufs=3, name=f"sp{qi}_{kk}")
                nc.tensor.matmul(sp[:], lhsT=khat[:, kt * P:(kt + 1) * P],
                                 rhs=qs4, start=True, stop=True)
                pt = aw.tile([P, GH * P], f16, tag="ptq")
                nc.scalar.activation(pt[:], sp[:], AF.Exp, bias=nexp_b[:])
                masked = (kk == 0) or (kk == WT and ktc == WT + 1)
                if masked:
                    msk = mlo_sb if kk == 0 else mhi_sb
                    nc.vector.scalar_tensor_tensor(
                        out=pt[:], in0=pt[:], scalar=1.0, in1=msk[:],
                        op0=ALU.mult, op1=ALU.mult)
                # denominator accumulation (fp16, 4x TSP mode)
                if kk == 0:
                    pacc = aw.tile([P, 512], f16, tag="pacc", bufs=2,
                                   name=f"pacc{qi}")
                    pacc_of[qi] = pacc
                    # init copy in 4x TSP form: pt is already masked, and the
                    # mask is 0/1 so multiplying by it again is a no-op
                    nc.vector.scalar_tensor_tensor(
                        out=pacc[:], in0=pt[:], scalar=1.0, in1=mlo_sb[:],
                        op0=ALU.mult, op1=ALU.mult)
                else:
                    pacc = pacc_of[qi]
                    nc.vector.scalar_tensor_tensor(
                        out=pacc[:], in0=pacc[:], scalar=1.0, in1=pt[:],
                        op0=ALU.mult, op1=ALU.add)
                return pt

            def attn_pv(qi, kk, pt):
                ktc = min(WT + 1, NT - qi)
                kt = qi + kk
                if kk == 0:
                    yps_of[qi] = pp.tile([P, GH * P], f32, tag="yps", bufs=2,
                                         name=f"yp{qi}")
                nc.tensor.matmul(yps_of[qi][:], lhsT=vsb[:, kt, :], rhs=pt[:],
                                 start=(kk == 0), stop=(kk == ktc - 1))

            def attn_out(qi):
                qsl = slice(qi * P, (qi + 1) * P)
                if debug:
                    nc.sync.dma_start(out=pacc_d[:, qi, :], in_=pacc_of[qi][:])
                denp = pp.tile([1, 512], f32, tag="sp", bufs=3, name=f"den{qi}")
                nc.tensor.matmul(denp[:], lhsT=ones_sb[:], rhs=pacc_of[qi][:],
                                 start=True, stop=True)
                rd = tl.tile([1, 512], f32, tag="rd", bufs=2)
                nc.vector.reciprocal_approx_fast(rd[:], denp[:])
                rd16 = tl.tile([1, 512], f16, tag="rd16", bufs=2)
                nc.scalar.copy(rd16[:], rd[:])
                rdb = pp.tile([P, 512], f32, tag="sp", bufs=3, name=f"rdb{qi}")
                nc.tensor.matmul(rdb[:], lhsT=ones1h[:], rhs=rd16[:],
                                 start=True, stop=True)
                yut = op_pool.tile([P, GH * P], f16, tag="yut", bufs=2)
                nc.scalar.copy(yut[:], yps_of[qi][:])   # frees yps
                yq = op_pool.tile([P, GH * P], f16, tag="yq", bufs=2)
                nc.vector.tensor_mul(yq[:], yut[:], rdb[:])
                for co in range(CO):
                    osl = slice(co * 512, co * 512 + 512)
                    ops = pp.tile([P, 512], f32, tag="ops", bufs=2,
                                  name=f"ops{qi}_{co}")
                    for h in range(GH):
                        nc.tensor.matmul(
                            ops[:], lhsT=yq[:, h * P:(h + 1) * P],
                            rhs=wo_sb[:, h, osl],
                            start=(h == 0), stop=(h == GH - 1))
                    ob = op_pool.tile([P, 512], f16, tag="ob")
                    nc.vector.tensor_copy(out=ob[:], in_=ops[:])
                    nc.sync.dma_start(out=out_d[qsl, osl], in_=ob[:])

            if debug:
                nc.sync.dma_start(out=khat_d[:], in_=khat[:])
                nc.sync.dma_start(out=qhat_d[:], in_=qhat[:])
                nc.sync.dma_start(out=vsb_d[:], in_=vsb[:])
                nc.sync.dma_start(out=sig_d[:], in_=sig_sb[:])

            # software pipeline: PV trails scores by 2 steps; the out
            # projection of row qi-1 is emitted a few steps into row qi so
            # its den-chain has drained by then.
            pv_queue = deque()
            done_out = set()
            out_ready = deque()
            for qi in range(NT):
                ktc = min(WT + 1, NT - qi)
                for kk in range(ktc):
                    pt = attn_step(qi, kk)
                    if len(pv_queue) >= 2:
                        attn_pv(*pv_queue.popleft())
                    pv_queue.append((qi, kk, pt))
                    if out_ready and out_ready[0][1] <= 0:
                        done_out.add(out_ready[0][0])
                        attn_out(out_ready.popleft()[0])
                    out_ready = deque([(q, age - 1) for q, age in out_ready])
                    if qi > 0 and (qi - 1) not in done_out \
                            and all(q != qi - 1 for q, _ in out_ready) \
                            and not any(q == qi - 1 for q, _, _ in pv_queue):
                        out_ready.append((qi - 1, 1))
            while pv_queue:
                attn_pv(*pv_queue.popleft())
            for qi in range(NT):
                if qi not in done_out:
                    attn_out(qi)

    return nc


def _get_program(T_=T, C_=C, win=WINDOW):
    key = (T_, C_, win)
    if key not in _PROGRAM_CACHE:
        nc = build_program(T_, C_, win)
        nc.finalize()
        _PROGRAM_CACHE[key] = nc
    return _PROGRAM_CACHE[key]


def make_in_maps(x, ve, cos, sin, Wq, Wk, Wv, Wg, Wo):
    """Build the 8 per-core input dicts (host-side sharding/layout prep)."""
    cosT = np.ascontiguousarray(cos[:, 0, :].T).astype(np.float32)  # [64, T]
    sinT = np.ascontiguousarray(sin[:, 0, :].T).astype(np.float32)
    cc = np.concatenate([cosT, cosT], axis=0)            # [128, T]
    ss = np.concatenate([sinT, -sinT], axis=0)           # [128, T]
    # 0/1 multiplicative band masks for the S^T diagonal/far tiles,
    # replicated across the 4 fused q heads
    k_idx = np.arange(P)[:, None]
    q_idx = np.arange(P)[None, :]
    mlo = np.tile((k_idx >= q_idx).astype(np.float32), (1, GH)).astype(F16)
    mhi = np.tile((k_idx < q_idx).astype(np.float32), (1, GH)).astype(F16)

    in_maps = []
    for core in range(N_CORES):
        b, g = divmod(core, N_KV)
        in_maps.append({
            "xT": np.ascontiguousarray(x[b].T).astype(F16),
            "wq": Wq[:, g * GH * HD:(g + 1) * GH * HD].astype(F16),
            "wk": Wk[:, g * HD:(g + 1) * HD].astype(F16),
            "wv": Wv[:, g * HD:(g + 1) * HD].astype(F16),
            "wg": np.ascontiguousarray(Wg[:, g:g + 1]).astype(F16),
            "ve2": (2.0 * ve[b][:, g * HD:(g + 1) * HD]).astype(F16),
            "wo": Wo[g * GH * HD:(g + 1) * GH * HD, :].astype(F16),
            "cc": cc.astype(F16), "ss": ss.astype(F16),
            "mlo": mlo, "mhi": mhi,
        })
    return in_maps


def kernel(x, ve, cos, sin, Wq, Wk, Wv, Wg, Wo, window):
    assert int(window) == WINDOW and x.shape == (B, T, C)
    from concourse.bass_utils import run_bass_kernel_spmd

    nc = _get_program()
    in_maps = make_in_maps(x, ve, cos, sin, Wq, Wk, Wv, Wg, Wo)
    res = run_bass_kernel_spmd(nc, in_maps, core_ids=list(range(N_CORES)))
    out = np.zeros((B, T, C), dtype=np.float32)
    for core in range(N_CORES):
        b = core // N_KV
        out[b] += res.results[core]["out"].astype(np.float32)
    return out


# revision 22
# speedup vs baseline: 1.0795x; 1.0161x over previous
"""Trainium2 Bass kernel for a GQA sliding-window attention layer.

Reference computation (B=2, T=2048, C=2048, 16 Q heads / 4 KV heads, d=128):
    q = x @ Wq; k = x @ Wk; v = x @ Wv (+ sigmoid-gated value embedding)
    q, k = rmsnorm(rope(q)), rmsnorm(rope(k))
    scores masked to the band 0 <= j - i < window (=1024), softmax over j
    out = (p @ v) @ Wo

Sharding: 8 cores = 2 batches x 4 KV groups.  Each core computes its 4 Q
heads / 1 KV head for one batch and a partial output (its 512-row slice of
the Wo contraction); the host sums the 4 partials per batch.

Key implementation notes:
  - fp16 everywhere (same PE/DVE speed as bf16, 8x the mantissa bits).
  - every PE matmul moves fp16 at 1 cycle/row; the only non-GEMM PE work
    is tiny [1,512] row-sums (rms ssq, softmax den) and their fp16
    [128,512] broadcast matmuls (213ns each at full clock).
  - band masks are 0/1 fp16 multiplies on the DVE (4x mode via
    scalar_tensor_tensor) applied to exp(scores) tiles.
  - softmax denominator: fp16 DVE accumulation of the exp tiles, then one
    row-sum matmul per 128-query row.
  - rope reads the projection PSUM directly (the half-swap addressing is
    only legal with a PSUM operand); the swapped-half multiplies run on
    the otherwise-idle Pool engine.
  - gate sigmoid is computed via Exp (1/(1+e^-x)) at the start of the
    attention phase so the ACT engine needs only one table set per phase
    (Sqrt during projection, Exp afterwards).
  - exp has a -2.0 bias folded in (cancels in the softmax ratio) so fp16
    can't overflow even for adversarially aligned q/k.
  - x is DMA'd in token-slice-major order so the first projection wave
    starts after ~1/16 of x has arrived; out is written as fp16.
"""

import numpy as np
from collections import deque

F16 = np.float16

# Problem dims (hardcoded per contest rules)
B, T, C = 2, 2048, 2048
N_HEAD, N_KV, HD, GATE_CH = 16, 4, 128, 32
WINDOW = 1024
P = 128
GH = N_HEAD // N_KV  # q heads per kv head (= per core)
N_CORES = 8

_PROGRAM_CACHE = {}


def build_program(T_=T, C_=C, win=WINDOW, debug=False):
    import concourse.mybir as mybir
    import concourse.tile as tile
    from concourse import bacc

    dt = mybir.dt
    f32 = dt.float32
    f16 = dt.float16
    AF = mybir.ActivationFunctionType
    ALU = mybir.AluOpType

    NT = T_ // P          # token tiles
    KT = C_ // P          # contraction tiles
    WT = win // P         # window tiles
    TS = T_ // 512        # 512-token slices

    nc = bacc.Bacc()

    xT = nc.declare_dram_parameter("xT", [C_, T_], f16, isOutput=False)
    wq = nc.declare_dram_parameter("wq", [C_, GH * HD], f16, isOutput=False)
    wk = nc.declare_dram_parameter("wk", [C_, HD], f16, isOutput=False)
    wv = nc.declare_dram_parameter("wv", [C_, HD], f16, isOutput=False)
    wg = nc.declare_dram_parameter("wg", [GATE_CH, 1], f16, isOutput=False)
    ve2 = nc.declare_dram_parameter("ve2", [T_, HD], f16, isOutput=False)
    wo = nc.declare_dram_parameter("wo", [GH * HD, C_], f16, isOutput=False)
    ccd = nc.declare_dram_parameter("cc", [P, T_], f16, isOutput=False)
    ssd = nc.declare_dram_parameter("ss", [P, T_], f16, isOutput=False)
    mlod = nc.declare_dram_parameter("mlo", [P, GH * P], f16, isOutput=False)
    mhid = nc.declare_dram_parameter("mhi", [P, GH * P], f16, isOutput=False)
    out_d = nc.declare_dram_parameter("out", [T_, C_], f16, isOutput=True)
    if debug:
        NTd = T_ // P
        khat_d = nc.declare_dram_parameter("khat_d", [P, T_], f16, isOutput=True)
        qhat_d = nc.declare_dram_parameter("qhat_d", [P, N_HEAD // N_KV, T_], f16, isOutput=True)
        vsb_d = nc.declare_dram_parameter("vsb_d", [P, NTd, HD], f16, isOutput=True)
        sig_d = nc.declare_dram_parameter("sig_d", [P, NTd], f32, isOutput=True)
        pacc_d = nc.declare_dram_parameter("pacc_d", [P, NTd, 512], f16, isOutput=True)

    with tile.TileContext(nc) as tc:
        with (
            tc.tile_pool(name="singles", bufs=1) as sg,
            tc.tile_pool(name="tails", bufs=3) as tl,
            tc.tile_pool(name="attn", bufs=4) as aw,
            tc.tile_pool(name="outp", bufs=3) as op_pool,
            tc.tile_pool(name="psum", bufs=1, space="PSUM") as pp,
        ):
            # ---- input DMAs --------------------------------------------
            wg_sb = sg.tile([GATE_CH, 1], f16, tag="wg")
            nc.sync.dma_start(out=wg_sb[:], in_=wg[:])
            wq_sb = sg.tile([P, KT, GH * HD], f16, tag="wq")
            wk_sb = sg.tile([P, KT, HD], f16, tag="wk")
            wv_sb = sg.tile([P, KT, HD], f16, tag="wv")
            wqr = wq.rearrange("(o p) n -> p o n", p=P)
            wkr = wk.rearrange("(o p) n -> p o n", p=P)
            wvr = wv.rearrange("(o p) n -> p o n", p=P)
            xt = []
            for kt in range(KT):
                t_ = sg.tile([P, T_], f16, tag=f"xt{kt}", name=f"xt{kt}")
                xt.append(t_)
            # whole-tensor weight DMAs (each dma_start costs ~0.6us of sync
            # engine issue time, so fewer + bigger wins), then x in
            # half-slices so the first waves can chase the stream.
            nc.sync.dma_start(out=wk_sb[:], in_=wkr[:])
            nc.sync.dma_start(out=wq_sb[:], in_=wqr[:])
            nc.sync.dma_start(out=wv_sb[:], in_=wvr[:])
            HT = T_ // 2
            for kt in range(KT):
                nc.sync.dma_start(out=xt[kt][:, 0:HT],
                                  in_=xT[kt * P:(kt + 1) * P, 0:HT])
            # constants needed by the first wave's tails
            cc_sb = sg.tile([P, T_], f16, tag="cc")
            nc.sync.dma_start(out=cc_sb[:], in_=ccd[:])
            ss_sb = sg.tile([P, T_], f16, tag="ss")
            nc.sync.dma_start(out=ss_sb[:], in_=ssd[:])
            ve2_sb = sg.tile([P, NT, HD], f16, tag="ve2")
            nc.sync.dma_start(out=ve2_sb[:], in_=ve2.rearrange("(o p) d -> p o d", p=P))
            mlo_sb = sg.tile([P, GH * P], f16, tag="mlo")
            nc.sync.dma_start(out=mlo_sb[:], in_=mlod[:])
            mhi_sb = sg.tile([P, GH * P], f16, tag="mhi")
            nc.sync.dma_start(out=mhi_sb[:], in_=mhid[:])
            for kt in range(KT):
                nc.sync.dma_start(out=xt[kt][:, HT:T_],
                                  in_=xT[kt * P:(kt + 1) * P, HT:T_])
            wo_sb = sg.tile([P, GH, C_], f16, tag="wo")
            nc.sync.dma_start(out=wo_sb[:], in_=wo.rearrange("(o p) n -> p o n", p=P))

            ones_sb = sg.tile([P, 1], f16, tag="onesb")
            nc.vector.memset(ones_sb[:], 1.0)
            ones1h = sg.tile([1, P], f16, tag="ones1h")
            nc.vector.memset(ones1h[:], 1.0)
            eps_sb = sg.tile([P, 1], f32, tag="epsb")
            nc.vector.memset(eps_sb[:], 1e-6)
            # k gets the 1/sqrt(d) score scale folded into its rms scale:
            # rr_k = (1/sqrt(ssq/HD+eps))/sqrt(HD) = 1/sqrt(ssq + HD*eps)
            epsk_sb = sg.tile([P, 1], f32, tag="epskb")
            nc.vector.memset(epsk_sb[:], HD * 1e-6)
            nexp_b = sg.tile([P, 1], f32, tag="nexpb")
            nc.vector.memset(nexp_b[:], -2.0)

            # persistent intermediates
            qhat = sg.tile([P, GH, T_], f16, tag="qhat")   # normalized roped q, [d, h, t]
            khat = sg.tile([P, T_], f16, tag="khat")       # normalized roped k * isq
            vsb = sg.tile([P, NT, HD], f16, tag="vsb")     # v (gated during attn start)
            sig_sb = sg.tile([P, NT], f32, tag="sigsb")    # gate sigmoids

            # ---- projection phase --------------------------------------
            gps = pp.tile([P, NT], f32, tag="gps", bufs=1)

            def wave_mms(ts, heads, tags):
                sl = slice(ts * 512, ts * 512 + 512)
                items = []
                for head, tag in zip(heads, tags):
                    nb = {"sp": 3, "yps": 2, "ops": 2}[tag]
                    if head == GH + 1:  # V group, [tok, d] per token tile
                        ps = pp.tile([P, 4, HD], f32, tag=tag, bufs=nb,
                                     name=f"psv{ts}")
                    else:
                        ps = pp.tile([P, 512], f32, tag=tag, bufs=nb,
                                     name=f"ps{head}_{ts}")
                    items.append((head, ts, ps))
                for kt in range(KT):
                    for head, _, ps in items:
                        if head == GH + 1:
                            continue
                        if head == 0:
                            nc.tensor.matmul(
                                ps[:], lhsT=wk_sb[:, kt, :],
                                rhs=xt[kt][:, sl],
                                start=(kt == 0), stop=(kt == KT - 1))
                        else:
                            nc.tensor.matmul(
                                ps[:], lhsT=wq_sb[:, kt, (head - 1) * HD:head * HD],
                                rhs=xt[kt][:, sl],
                                start=(kt == 0), stop=(kt == KT - 1))
                # the V chains share one psum bank, so a chain's start=True
                # would mark the whole bank pending-zero and clobber any
                # other open chain's first term: run them strictly one at a
                # time (j outer, kt inner).
                for head, _, ps in items:
                    if head != GH + 1:
                        continue
                    for j in range(4):
                        tsl = slice(ts * 512 + j * P, ts * 512 + (j + 1) * P)
                        for kt in range(KT):
                            nc.tensor.matmul(
                                ps[:, j, :], lhsT=xt[kt][:, tsl],
                                rhs=wv_sb[:, kt, :],
                                start=(kt == 0), stop=(kt == KT - 1),
                                skip_group_check=True)
                if heads[-1] == GH + 1:
                    for j in range(4):
                        tt = ts * 4 + j
                        tsl = slice(tt * P, (tt + 1) * P)
                        nc.tensor.matmul(
                            gps[:, tt:tt + 1], lhsT=xt[0][0:GATE_CH, tsl],
                            rhs=wg_sb[:], start=True, stop=True,
                            skip_group_check=True)
                return items

            def tails(items):
                # 1) ropes: free the projection psums first.  The
                #    half-swapped multiplies read the PSUM (partition cross
                #    is only legal there) and run on the Pool engine.
                t1 = []
                for head, ts, ps in items:
                    if head == GH + 1:
                        # raw v copy [tok, d] -> SBUF (gating happens later)
                        nc.scalar.copy(vsb[:, ts * 4:(ts + 1) * 4, :], ps[:])
                        continue
                    sl = slice(ts * 512, ts * 512 + 512)
                    qr = tl.tile([P, 512], f16, tag="qr", bufs=5)
                    nc.vector.tensor_mul(qr[:], ps[:], cc_sb[:, sl])
                    qs = tl.tile([P, 512], f16, tag="qs", bufs=5)
                    nc.vector.tensor_mul(qs[0:64, :], ps[64:128, :], ss_sb[0:64, sl])
                    nc.vector.tensor_mul(qs[64:128, :], ps[0:64, :], ss_sb[64:128, sl])
                    # scalar as fp16 [P,1] AP: an f32 immediate would disable
                    # the DVE 2-byte fast modes (4x) in the cost model
                    nc.vector.scalar_tensor_tensor(
                        out=qr[:], in0=qr[:], scalar=ones_sb[:], in1=qs[:],
                        op0=ALU.mult, op1=ALU.add)
                    t1.append((head, sl, qr))
                # 2) squares on Pool
                q2s = []
                for head, sl, qr in t1:
                    q2 = tl.tile([P, 512], f16, tag="q2", bufs=5)
                    nc.gpsimd.tensor_mul(q2[:], qr[:], qr[:])
                    q2s.append(q2)
                # 3) per group: row-sum matmul, sqrt, recip, cast, broadcast
                #    matmul, final scale.  sp-tag psums rotate: each is freed
                #    by the fast ACT/DVE op right behind it.
                for (head, sl, qr), q2 in zip(t1, q2s):
                    ssq = pp.tile([1, 512], f32, tag="sp", bufs=3, name="ssq")
                    nc.tensor.matmul(ssq[:], lhsT=ones_sb[:], rhs=q2[:],
                                     start=True, stop=True)
                    sq = tl.tile([1, 512], f32, tag="sq", bufs=4)
                    if head == 0:
                        nc.scalar.activation(sq[:], ssq[:], AF.Sqrt,
                                             bias=epsk_sb[0:1, :], scale=1.0)
                    else:
                        nc.scalar.activation(sq[:], ssq[:], AF.Sqrt,
                                             bias=eps_sb[0:1, :], scale=1.0 / HD)
                    rr = tl.tile([1, 512], f32, tag="rr", bufs=4)
                    nc.vector.reciprocal_approx_fast(rr[:], sq[:])
                    rr16 = tl.tile([1, 512], f16, tag="rr16", bufs=4)
                    nc.scalar.copy(rr16[:], rr[:])
                    rrb = pp.tile([P, 512], f32, tag="sp", bufs=3, name="rrb")
                    nc.tensor.matmul(rrb[:], lhsT=ones1h[:], rhs=rr16[:],
                                     start=True, stop=True)
                    dest = khat[:, sl] if head == 0 else qhat[:, head - 1, sl]
                    nc.vector.tensor_mul(dest, qr[:], rrb[:])

            for ts in range(TS):
                items_a = wave_mms(ts, [0, 1, 2], ["sp", "sp", "sp"])
                items_b = wave_mms(ts, [3, 4, GH + 1], ["yps", "yps", "ops"])
                tails(items_a)
                tails(items_b)

            # ---- gate sigmoids + v gating (one Exp table load) ---------
            egate = tl.tile([P, NT], f32, tag="egate", bufs=1)
            nc.scalar.activation(egate[:], gps[:], AF.Exp, scale=-1.0)  # e^-g
            ep1 = tl.tile([P, NT], f32, tag="ep1", bufs=1)
            nc.vector.tensor_scalar_add(ep1[:], egate[:], 1.0)
            nc.vector.reciprocal_approx_fast(sig_sb[:], ep1[:])         # sigmoid
            for tt in range(NT):
                # v += sigmoid * ve2 (ve2 pre-scaled by 2), in place
                nc.vector.scalar_tensor_tensor(
                    out=vsb[:, tt, :], in0=ve2_sb[:, tt, :],
                    scalar=sig_sb[:, tt:tt + 1],
                    in1=vsb[:, tt, :], op0=ALU.mult, op1=ALU.add)

            # ---- attention + out-projection ----------------------------
            CO = C_ // 512  # output column chunks

            yps_of = {}
            pacc_of = {}

            def attn_step(qi, kk):
                ktc = min(WT + 1, NT - qi)
                kt = qi + kk
                qs4 = qhat[:, :, qi * P:(qi + 1) * P]   # [d, (h, q)] = 512 wide
                sp = pp.tile([P, 512], f32, tag="sp", bufs=3, name=f"sp{qi}_{kk}")
                nc.tensor.matmul(sp[:], lhsT=khat[:, kt * P:(kt + 1) * P],
                                 rhs=qs4, start=True, stop=True)
                pt = aw.tile([P, GH * P], f16, tag="ptq", bufs=6)
                nc.scalar.activation(pt[:], sp[:], AF.Exp, bias=nexp_b[:])
                masked = (kk == 0) or (kk == WT and ktc == WT + 1)
                if masked:
                    msk = mlo_sb if kk == 0 else mhi_sb
                    nc.vector.scalar_tensor_tensor(
                        out=pt[:], in0=pt[:], scalar=ones_sb[:], in1=msk[:],
                        op0=ALU.mult, op1=ALU.mult)
                # denominator accumulation (fp16, 4x TSP mode)
                if kk == 0:
                    pacc = aw.tile([P, 512], f16, tag="pacc", bufs=3,
                                   name=f"pacc{qi}")
                    pacc_of[qi] = pacc
                    # init copy in 4x TSP form: pt is already masked, and the
                    # mask is 0/1 so multiplying by it again is a no-op
                    nc.vector.scalar_tensor_tensor(
                        out=pacc[:], in0=pt[:], scalar=ones_sb[:], in1=mlo_sb[:],
                        op0=ALU.mult, op1=ALU.mult)
                else:
                    pacc = pacc_of[qi]
                    nc.vector.scalar_tensor_tensor(
                        out=pacc[:], in0=pacc[:], scalar=ones_sb[:], in1=pt[:],
                        op0=ALU.mult, op1=ALU.add)
                return pt

            def attn_pv(qi, kk, pt):
                ktc = min(WT + 1, NT - qi)
                kt = qi + kk
                if kk == 0:
                    yps_of[qi] = pp.tile([P, GH * P], f32, tag="yps", bufs=2,
                                         name=f"yp{qi}")
                nc.tensor.matmul(yps_of[qi][:], lhsT=vsb[:, kt, :], rhs=pt[:],
                                 start=(kk == 0), stop=(kk == ktc - 1))

            rd16_of = {}
            yq_of = {}

            def attn_out_a(qi):
                # denominator row-sum -> reciprocal -> fp16 cast
                if debug:
                    nc.sync.dma_start(out=pacc_d[:, qi, :], in_=pacc_of[qi][:])
                denp = pp.tile([1, 512], f32, tag="sp", bufs=3, name=f"den{qi}")
                nc.tensor.matmul(denp[:], lhsT=ones_sb[:], rhs=pacc_of[qi][:],
                                 start=True, stop=True)
                rd = tl.tile([1, 512], f32, tag="rd", bufs=2)
                nc.vector.reciprocal_approx_fast(rd[:], denp[:])
                rd16 = tl.tile([1, 512], f16, tag="rd16", bufs=2)
                nc.scalar.copy(rd16[:], rd[:])
                rd16_of[qi] = rd16

            def attn_out_b(qi):
                # broadcast + normalize
                rdb = pp.tile([P, 512], f32, tag="sp", bufs=3, name=f"rdb{qi}")
                nc.tensor.matmul(rdb[:], lhsT=ones1h[:], rhs=rd16_of[qi][:],
                                 start=True, stop=True)
                yut = op_pool.tile([P, GH * P], f16, tag="yut", bufs=2)
                nc.scalar.copy(yut[:], yps_of[qi][:])   # frees yps
                yq = op_pool.tile([P, GH * P], f16, tag="yq", bufs=2)
                nc.vector.tensor_mul(yq[:], yut[:], rdb[:])
                yq_of[qi] = yq

            def attn_out_c(qi):
                qsl = slice(qi * P, (qi + 1) * P)
                yq = yq_of[qi]
                ob = op_pool.tile([P, C_], f16, tag="ob", bufs=2)
                for co in range(CO):
                    osl = slice(co * 512, co * 512 + 512)
                    ops = pp.tile([P, 512], f32, tag="ops", bufs=2,
                                  name=f"ops{qi}_{co}")
                    for h in range(GH):
                        nc.tensor.matmul(
                            ops[:], lhsT=yq[:, h * P:(h + 1) * P],
                            rhs=wo_sb[:, h, osl],
                            start=(h == 0), stop=(h == GH - 1))
                    nc.vector.tensor_copy(out=ob[:, osl], in_=ops[:])
                nc.sync.dma_start(out=out_d[qsl, :], in_=ob[:])

            if debug:
                nc.sync.dma_start(out=khat_d[:], in_=khat[:])
                nc.sync.dma_start(out=qhat_d[:], in_=qhat[:])
                nc.sync.dma_start(out=vsb_d[:], in_=vsb[:])
                nc.sync.dma_start(out=sig_d[:], in_=sig_sb[:])

            # software pipeline over global attention steps: PV trails the
            # scores/exp stream by 4 steps; each finished row's out chain is
            # emitted in three staggered stages so the PE never waits on the
            # recip/cast/broadcast latency.
            pv_queue = deque()
            actions = deque()   # (trigger_step, fn)
            gs = 0
            for qi in range(NT):
                ktc = min(WT + 1, NT - qi)
                for kk in range(ktc):
                    while actions and actions[0][0] <= gs:
                        actions.popleft()[1]()
                    pt = attn_step(qi, kk)
                    if len(pv_queue) >= 4:
                        attn_pv(*pv_queue.popleft())
                    pv_queue.append((qi, kk, pt))
                    gs += 1
                actions.append((gs + 1, (lambda q: lambda: attn_out_a(q))(qi)))
                actions.append((gs + 4, (lambda q: lambda: attn_out_b(q))(qi)))
                actions.append((gs + 7, (lambda q: lambda: attn_out_c(q))(qi)))
            while pv_queue:
                attn_pv(*pv_queue.popleft())
            while actions:
                actions.popleft()[1]()

    return nc


def _get_program(T_=T, C_=C, win=WINDOW):
    key = (T_, C_, win)
    if key not in _PROGRAM_CACHE:
        nc = build_program(T_, C_, win)
        nc.finalize()
        _PROGRAM_CACHE[key] = nc
    return _PROGRAM_CACHE[key]


def make_in_maps(x, ve, cos, sin, Wq, Wk, Wv, Wg, Wo):
    """Build the 8 per-core input dicts (host-side sharding/layout prep)."""
    cosT = np.ascontiguousarray(cos[:, 0, :].T).astype(np.float32)  # [64, T]
    sinT = np.ascontiguousarray(sin[:, 0, :].T).astype(np.float32)
    cc = np.concatenate([cosT, cosT], axis=0)            # [128, T]
    ss = np.concatenate([sinT, -sinT], axis=0)           # [128, T]
    # 0/1 multiplicative band masks for the S^T diagonal/far tiles,
    # replicated across the 4 fused q heads
    k_idx = np.arange(P)[:, None]
    q_idx = np.arange(P)[None, :]
    mlo = np.tile((k_idx >= q_idx).astype(np.float32), (1, GH)).astype(F16)
    mhi = np.tile((k_idx < q_idx).astype(np.float32), (1, GH)).astype(F16)

    in_maps = []
    for core in range(N_CORES):
        b, g = divmod(core, N_KV)
        in_maps.append({
            "xT": np.ascontiguousarray(x[b].T).astype(F16),
            "wq": Wq[:, g * GH * HD:(g + 1) * GH * HD].astype(F16),
            "wk": Wk[:, g * HD:(g + 1) * HD].astype(F16),
            "wv": Wv[:, g * HD:(g + 1) * HD].astype(F16),
            "wg": np.ascontiguousarray(Wg[:, g:g + 1]).astype(F16),
            "ve2": (2.0 * ve[b][:, g * HD:(g + 1) * HD]).astype(F16),
            "wo": Wo[g * GH * HD:(g + 1) * GH * HD, :].astype(F16),
            "cc": cc.astype(F16), "ss": ss.astype(F16),
            "mlo": mlo, "mhi": mhi,
        })
    return in_maps


def kernel(x, ve, cos, sin, Wq, Wk, Wv, Wg, Wo, window):
    assert int(window) == WINDOW and x.shape == (B, T, C)
    from concourse.bass_utils import run_bass_kernel_spmd

    nc = _get_program()
    in_maps = make_in_maps(x, ve, cos, sin, Wq, Wk, Wv, Wg, Wo)
    res = run_bass_kernel_spmd(nc, in_maps, core_ids=list(range(N_CORES)))
    out = np.zeros((B, T, C), dtype=np.float32)
    for core in range(N_CORES):
        b = core // N_KV
        out[b] += res.results[core]["out"].astype(np.float32)
    return out


# revision 24
# speedup vs baseline: 1.2273x; 1.1369x over previous
"""Trainium2 Bass kernel for a GQA sliding-window attention layer.

Reference computation (B=2, T=2048, C=2048, 16 Q heads / 4 KV heads, d=128):
    q = x @ Wq; k = x @ Wk; v = x @ Wv (+ sigmoid-gated value embedding)
    q, k = rmsnorm(rope(q)), rmsnorm(rope(k))
    scores masked to the band 0 <= j - i < window (=1024), softmax over j
    out = (p @ v) @ Wo

Sharding: 8 cores = 2 batches x 4 KV groups.  Each core computes its 4 Q
heads / 1 KV head for one batch and a partial output (its 512-row slice of
the Wo contraction); the host sums the 4 partials per batch.

Key implementation notes:
  - fp16 everywhere (same PE/DVE speed as bf16, 8x the mantissa bits).
  - every PE matmul moves fp16 at 1 cycle/row; the only non-GEMM PE work
    is tiny [1,512] row-sums (rms ssq, softmax den) and their fp16
    [128,512] broadcast matmuls (213ns each at full clock).
  - band masks are 0/1 fp16 multiplies on the DVE (4x mode via
    scalar_tensor_tensor) applied to exp(scores) tiles.
  - softmax denominator: fp16 DVE accumulation of the exp tiles, then one
    row-sum matmul per 128-query row.
  - rope reads the projection PSUM directly (the half-swap addressing is
    only legal with a PSUM operand); the swapped-half multiplies run on
    the otherwise-idle Pool engine.
  - gate sigmoid is computed via Exp (1/(1+e^-x)) at the start of the
    attention phase so the ACT engine needs only one table set per phase
    (Sqrt during projection, Exp afterwards).
  - exp has a -2.0 bias folded in (cancels in the softmax ratio) so fp16
    can't overflow even for adversarially aligned q/k.
  - x is DMA'd in token-slice-major order so the first projection wave
    starts after ~1/16 of x has arrived; out is written as fp16.
"""

import numpy as np
from collections import deque

F16 = np.float16

# Problem dims (hardcoded per contest rules)
B, T, C = 2, 2048, 2048
N_HEAD, N_KV, HD, GATE_CH = 16, 4, 128, 32
WINDOW = 1024
P = 128
GH = N_HEAD // N_KV  # q heads per kv head (= per core)
N_CORES = 8

_PROGRAM_CACHE = {}


def build_program(T_=T, C_=C, win=WINDOW, debug=False):
    import concourse.mybir as mybir
    import concourse.tile as tile
    from concourse import bacc

    dt = mybir.dt
    f32 = dt.float32
    f16 = dt.float16
    AF = mybir.ActivationFunctionType
    ALU = mybir.AluOpType

    NT = T_ // P          # token tiles
    KT = C_ // P          # contraction tiles
    WT = win // P         # window tiles
    TS = T_ // 512        # 512-token slices

    nc = bacc.Bacc()

    xT = nc.declare_dram_parameter("xT", [C_, T_], f16, isOutput=False)
    wq = nc.declare_dram_parameter("wq", [C_, GH * HD], f16, isOutput=False)
    wk = nc.declare_dram_parameter("wk", [C_, HD], f16, isOutput=False)
    wv = nc.declare_dram_parameter("wv", [C_, HD], f16, isOutput=False)
    wg = nc.declare_dram_parameter("wg", [GATE_CH, 1], f16, isOutput=False)
    ve2 = nc.declare_dram_parameter("ve2", [T_, HD], f16, isOutput=False)
    wo = nc.declare_dram_parameter("wo", [GH * HD, C_], f16, isOutput=False)
    ccd = nc.declare_dram_parameter("cc", [P, T_], f16, isOutput=False)
    ssd = nc.declare_dram_parameter("ss", [P, T_], f16, isOutput=False)
    mlod = nc.declare_dram_parameter("mlo", [P, GH * P], f16, isOutput=False)
    mhid = nc.declare_dram_parameter("mhi", [P, GH * P], f16, isOutput=False)
    out_d = nc.declare_dram_parameter("out", [T_, C_], f16, isOutput=True)
    if debug:
        NTd = T_ // P
        khat_d = nc.declare_dram_parameter("khat_d", [P, T_], f16, isOutput=True)
        qhat_d = nc.declare_dram_parameter("qhat_d", [P, N_HEAD // N_KV, T_], f16, isOutput=True)
        vsb_d = nc.declare_dram_parameter("vsb_d", [P, NTd, HD], f16, isOutput=True)
        sig_d = nc.declare_dram_parameter("sig_d", [P, NTd], f32, isOutput=True)
        pacc_d = nc.declare_dram_parameter("pacc_d", [P, NTd, 512], f16, isOutput=True)

    with tile.TileContext(nc) as tc:
        with (
            tc.tile_pool(name="singles", bufs=1) as sg,
            tc.tile_pool(name="tails", bufs=3) as tl,
            tc.tile_pool(name="attn", bufs=4) as aw,
            tc.tile_pool(name="outp", bufs=3) as op_pool,
            tc.tile_pool(name="psum", bufs=1, space="PSUM") as pp,
        ):
            # ---- input DMAs --------------------------------------------
            wg_sb = sg.tile([GATE_CH, 1], f16, tag="wg")
            nc.sync.dma_start(out=wg_sb[:], in_=wg[:])
            wq_sb = sg.tile([P, KT, GH * HD], f16, tag="wq")
            wk_sb = sg.tile([P, KT, HD], f16, tag="wk")
            wv_sb = sg.tile([P, KT, HD], f16, tag="wv")
            wqr = wq.rearrange("(o p) n -> p o n", p=P)
            wkr = wk.rearrange("(o p) n -> p o n", p=P)
            wvr = wv.rearrange("(o p) n -> p o n", p=P)
            xt = []
            for kt in range(KT):
                t_ = sg.tile([P, T_], f16, tag=f"xt{kt}", name=f"xt{kt}")
                xt.append(t_)
            # weights on the sync queue (wq split per head so wave A can
            # start early), x halves on the Pool queue in parallel (each
            # dma_start costs ~0.6us of issue time on its queue).
            nc.sync.dma_start(out=wk_sb[:], in_=wkr[:])
            for h in range(GH):
                nc.sync.dma_start(out=wq_sb[:, :, h * HD:(h + 1) * HD],
                                  in_=wqr[:, :, h * HD:(h + 1) * HD])
                if h == 1:
                    nc.sync.dma_start(out=wv_sb[:], in_=wvr[:])
            HT = T_ // 2
            for kt in range(KT):
                nc.gpsimd.dma_start(out=xt[kt][:, 0:HT],
                                    in_=xT[kt * P:(kt + 1) * P, 0:HT])
            # constants needed by the first wave's tails
            cc_sb = sg.tile([P, T_], f16, tag="cc")
            nc.sync.dma_start(out=cc_sb[:], in_=ccd[:])
            ss_sb = sg.tile([P, T_], f16, tag="ss")
            nc.sync.dma_start(out=ss_sb[:], in_=ssd[:])
            ve2_sb = sg.tile([P, NT, HD], f16, tag="ve2")
            nc.sync.dma_start(out=ve2_sb[:], in_=ve2.rearrange("(o p) d -> p o d", p=P))
            mlo_sb = sg.tile([P, GH * P], f16, tag="mlo")
            nc.sync.dma_start(out=mlo_sb[:], in_=mlod[:])
            mhi_sb = sg.tile([P, GH * P], f16, tag="mhi")
            nc.sync.dma_start(out=mhi_sb[:], in_=mhid[:])
            for kt in range(KT):
                nc.gpsimd.dma_start(out=xt[kt][:, HT:T_],
                                    in_=xT[kt * P:(kt + 1) * P, HT:T_])
            wo_sb = sg.tile([P, GH, C_], f16, tag="wo")
            nc.sync.dma_start(out=wo_sb[:], in_=wo.rearrange("(o p) n -> p o n", p=P))

            ones_sb = sg.tile([P, 1], f16, tag="onesb")
            nc.vector.memset(ones_sb[:], 1.0)
            ones1h = sg.tile([1, P], f16, tag="ones1h")
            nc.vector.memset(ones1h[:], 1.0)
            eps_sb = sg.tile([P, 1], f32, tag="epsb")
            nc.vector.memset(eps_sb[:], 1e-6)
            # k gets the 1/sqrt(d) score scale folded into its rms scale:
            # rr_k = (1/sqrt(ssq/HD+eps))/sqrt(HD) = 1/sqrt(ssq + HD*eps)
            epsk_sb = sg.tile([P, 1], f32, tag="epskb")
            nc.vector.memset(epsk_sb[:], HD * 1e-6)
            nexp_b = sg.tile([P, 1], f32, tag="nexpb")
            nc.vector.memset(nexp_b[:], -2.0)

            # persistent intermediates
            qhat = sg.tile([P, GH, T_], f16, tag="qhat")   # normalized roped q, [d, h, t]
            khat = sg.tile([P, T_], f16, tag="khat")       # normalized roped k * isq
            vsb = sg.tile([P, NT, HD], f16, tag="vsb")     # v (gated during attn start)
            sig_sb = sg.tile([P, NT], f32, tag="sigsb")    # gate sigmoids

            # ---- projection phase --------------------------------------
            # gate columns land in SBUF via a per-slice psum bounce (Copy is
            # in every ACT table set, so no table thrash)
            gcol_sb = sg.tile([P, NT], f32, tag="gcolsb")

            def wave_mms(ts, heads, tags):
                sl = slice(ts * 512, ts * 512 + 512)
                items = []
                for head, tag in zip(heads, tags):
                    nb = {"sp": 4, "yps": 2, "ops": 2}[tag]
                    if head == GH + 1:  # V group, [tok, d] per token tile
                        ps = pp.tile([P, 4, HD], f32, tag=tag, bufs=nb,
                                     name=f"psv{ts}")
                    else:
                        ps = pp.tile([P, 512], f32, tag=tag, bufs=nb,
                                     name=f"ps{head}_{ts}")
                    items.append((head, ts, ps))
                for kt in range(KT):
                    for head, _, ps in items:
                        if head == GH + 1:
                            continue
                        if head == 0:
                            nc.tensor.matmul(
                                ps[:], lhsT=wk_sb[:, kt, :],
                                rhs=xt[kt][:, sl],
                                start=(kt == 0), stop=(kt == KT - 1))
                        else:
                            nc.tensor.matmul(
                                ps[:], lhsT=wq_sb[:, kt, (head - 1) * HD:head * HD],
                                rhs=xt[kt][:, sl],
                                start=(kt == 0), stop=(kt == KT - 1))
                # the V chains share one psum bank, so a chain's start=True
                # would mark the whole bank pending-zero and clobber any
                # other open chain's first term: run them strictly one at a
                # time (j outer, kt inner).
                for head, _, ps in items:
                    if head != GH + 1:
                        continue
                    for j in range(4):
                        tsl = slice(ts * 512 + j * P, ts * 512 + (j + 1) * P)
                        for kt in range(KT):
                            nc.tensor.matmul(
                                ps[:, j, :], lhsT=xt[kt][:, tsl],
                                rhs=wv_sb[:, kt, :],
                                start=(kt == 0), stop=(kt == KT - 1),
                                skip_group_check=True)
                if heads[-1] == GH + 1:
                    gpsl = pp.tile([P, 4], f32, tag="sp", bufs=4,
                                   name=f"gps{ts}")
                    for j in range(4):
                        tsl = slice((ts * 4 + j) * P, (ts * 4 + j + 1) * P)
                        nc.tensor.matmul(
                            gpsl[:, j:j + 1], lhsT=xt[0][0:GATE_CH, tsl],
                            rhs=wg_sb[:], start=True, stop=True,
                            skip_group_check=True)
                    nc.scalar.copy(gcol_sb[:, ts * 4:(ts + 1) * 4], gpsl[:])
                return items

            def tails(items):
                # 1) ropes: free the projection psums first.  The
                #    half-swapped multiplies read the PSUM (partition cross
                #    is only legal there) and run on the Pool engine.
                t1 = []
                for head, ts, ps in items:
                    if head == GH + 1:
                        # raw v copy [tok, d] -> SBUF (gating happens later)
                        nc.scalar.copy(vsb[:, ts * 4:(ts + 1) * 4, :], ps[:])
                        continue
                    sl = slice(ts * 512, ts * 512 + 512)
                    qr = tl.tile([P, 512], f16, tag="qr", bufs=5)
                    nc.vector.tensor_mul(qr[:], ps[:], cc_sb[:, sl])
                    qs = tl.tile([P, 512], f16, tag="qs", bufs=5)
                    nc.vector.tensor_mul(qs[0:64, :], ps[64:128, :], ss_sb[0:64, sl])
                    nc.vector.tensor_mul(qs[64:128, :], ps[0:64, :], ss_sb[64:128, sl])
                    # plain tensor_tensor: the only DVE op class with a
                    # fast mode (2x_1p) for fp16 — STT has none
                    nc.vector.tensor_add(qr[:], qr[:], qs[:])
                    t1.append((head, sl, qr))
                # 2) squares on Pool
                q2s = []
                for head, sl, qr in t1:
                    q2 = tl.tile([P, 512], f16, tag="q2", bufs=5)
                    nc.gpsimd.tensor_mul(q2[:], qr[:], qr[:])
                    q2s.append(q2)
                # 3) per group: row-sum matmul, sqrt, recip, cast, broadcast
                #    matmul, final scale.  sp-tag psums rotate: each is freed
                #    by the fast ACT/DVE op right behind it.
                for (head, sl, qr), q2 in zip(t1, q2s):
                    ssq = pp.tile([1, 512], f32, tag="sp", bufs=4, name="ssq")
                    nc.tensor.matmul(ssq[:], lhsT=ones_sb[:], rhs=q2[:],
                                     start=True, stop=True)
                    sq = tl.tile([1, 512], f32, tag="sq", bufs=4)
                    if head == 0:
                        nc.scalar.activation(sq[:], ssq[:], AF.Sqrt,
                                             bias=epsk_sb[0:1, :], scale=1.0)
                    else:
                        nc.scalar.activation(sq[:], ssq[:], AF.Sqrt,
                                             bias=eps_sb[0:1, :], scale=1.0 / HD)
                    rr = tl.tile([1, 512], f32, tag="rr", bufs=4)
                    nc.vector.reciprocal_approx_fast(rr[:], sq[:])
                    rr16 = tl.tile([1, 512], f16, tag="rr16", bufs=4)
                    nc.scalar.copy(rr16[:], rr[:])
                    rrb = pp.tile([P, 512], f32, tag="sp", bufs=4, name="rrb")
                    nc.tensor.matmul(rrb[:], lhsT=ones1h[:], rhs=rr16[:],
                                     start=True, stop=True)
                    dest = khat[:, sl] if head == 0 else qhat[:, head - 1, sl]
                    nc.vector.tensor_mul(dest, qr[:], rrb[:])

            for ts in range(TS):
                items_a = wave_mms(ts, [0, 1, 2], ["sp", "sp", "sp"])
                items_b = wave_mms(ts, [3, 4, GH + 1], ["yps", "yps", "ops"])
                tails(items_a)
                tails(items_b)

            # ---- gate sigmoids + v gating (one Exp table load) ---------
            egate = tl.tile([P, NT], f32, tag="egate", bufs=1)
            nc.scalar.activation(egate[:], gcol_sb[:], AF.Exp, scale=-1.0)  # e^-g
            ep1 = tl.tile([P, NT], f32, tag="ep1", bufs=1)
            nc.vector.tensor_scalar_add(ep1[:], egate[:], 1.0)
            nc.vector.reciprocal_approx_fast(sig_sb[:], ep1[:])         # sigmoid
            for tt in range(NT):
                # v += sigmoid * ve2 (ve2 pre-scaled by 2), in place
                nc.vector.scalar_tensor_tensor(
                    out=vsb[:, tt, :], in0=ve2_sb[:, tt, :],
                    scalar=sig_sb[:, tt:tt + 1],
                    in1=vsb[:, tt, :], op0=ALU.mult, op1=ALU.add)

            # ---- attention + out-projection ----------------------------
            CO = C_ // 512  # output column chunks

            yps_of = {}
            pacc_of = {}

            def attn_step(qi, kk):
                ktc = min(WT + 1, NT - qi)
                kt = qi + kk
                qs4 = qhat[:, :, qi * P:(qi + 1) * P]   # [d, (h, q)] = 512 wide
                sp = pp.tile([P, 512], f32, tag="sp", bufs=4, name=f"sp{qi}_{kk}")
                nc.tensor.matmul(sp[:], lhsT=khat[:, kt * P:(kt + 1) * P],
                                 rhs=qs4, start=True, stop=True)
                pt = aw.tile([P, GH * P], f16, tag="ptq", bufs=6)
                nc.scalar.activation(pt[:], sp[:], AF.Exp, bias=nexp_b[:])
                masked = (kk == 0) or (kk == WT and ktc == WT + 1)
                if masked:
                    msk = mlo_sb if kk == 0 else mhi_sb
                    nc.vector.tensor_mul(pt[:], pt[:], msk[:])
                # denominator accumulation (fp16, 4x TSP mode)
                if kk == 0:
                    pacc = aw.tile([P, 512], f16, tag="pacc", bufs=3,
                                   name=f"pacc{qi}")
                    pacc_of[qi] = pacc
                    # init as a mul: pt is already masked and the mask is
                    # 0/1, so multiplying by it again is a no-op copy
                    nc.vector.tensor_mul(pacc[:], pt[:], mlo_sb[:])
                else:
                    pacc = pacc_of[qi]
                    nc.vector.tensor_add(pacc[:], pacc[:], pt[:])
                return pt

            def attn_pv(qi, kk, pt):
                ktc = min(WT + 1, NT - qi)
                kt = qi + kk
                if kk == 0:
                    yps_of[qi] = pp.tile([P, GH * P], f32, tag="yps", bufs=2,
                                         name=f"yp{qi}")
                nc.tensor.matmul(yps_of[qi][:], lhsT=vsb[:, kt, :], rhs=pt[:],
                                 start=(kk == 0), stop=(kk == ktc - 1))

            rd16_of = {}
            yq_of = {}

            def attn_out_a(qi):
                # denominator row-sum -> reciprocal -> fp16 cast
                if debug:
                    nc.sync.dma_start(out=pacc_d[:, qi, :], in_=pacc_of[qi][:])
                denp = pp.tile([1, 512], f32, tag="sp", bufs=4, name=f"den{qi}")
                nc.tensor.matmul(denp[:], lhsT=ones_sb[:], rhs=pacc_of[qi][:],
                                 start=True, stop=True)
                rd = tl.tile([1, 512], f32, tag="rd", bufs=2)
                nc.vector.reciprocal_approx_fast(rd[:], denp[:])
                rd16 = tl.tile([1, 512], f16, tag="rd16", bufs=2)
                nc.scalar.copy(rd16[:], rd[:])
                rd16_of[qi] = rd16

            def attn_out_b(qi):
                # broadcast + normalize
                rdb = pp.tile([P, 512], f32, tag="sp", bufs=4, name=f"rdb{qi}")
                nc.tensor.matmul(rdb[:], lhsT=ones1h[:], rhs=rd16_of[qi][:],
                                 start=True, stop=True)
                yut = op_pool.tile([P, GH * P], f16, tag="yut", bufs=2)
                nc.vector.tensor_copy(yut[:], yps_of[qi][:])   # frees yps
                yq = op_pool.tile([P, GH * P], f16, tag="yq", bufs=2)
                nc.vector.tensor_mul(yq[:], yut[:], rdb[:])
                yq_of[qi] = yq

            def attn_out_c(qi):
                qsl = slice(qi * P, (qi + 1) * P)
                yq = yq_of[qi]
                ob = op_pool.tile([P, C_], f16, tag="ob", bufs=2)
                for co in range(CO):
                    osl = slice(co * 512, co * 512 + 512)
                    ops = pp.tile([P, 512], f32, tag="ops", bufs=2,
                                  name=f"ops{qi}_{co}")
                    for h in range(GH):
                        nc.tensor.matmul(
                            ops[:], lhsT=yq[:, h * P:(h + 1) * P],
                            rhs=wo_sb[:, h, osl],
                            start=(h == 0), stop=(h == GH - 1))
                    if co % 2 == 0:
                        nc.scalar.copy(ob[:, osl], ops[:])
                    else:
                        nc.vector.tensor_copy(out=ob[:, osl], in_=ops[:])
                nc.sync.dma_start(out=out_d[qsl, :], in_=ob[:])

            if debug:
                nc.sync.dma_start(out=khat_d[:], in_=khat[:])
                nc.sync.dma_start(out=qhat_d[:], in_=qhat[:])
                nc.sync.dma_start(out=vsb_d[:], in_=vsb[:])
                nc.sync.dma_start(out=sig_d[:], in_=sig_sb[:])

            # software pipeline over global attention steps: PV trails the
            # scores/exp stream by 4 steps; each finished row's out chain is
            # emitted in three staggered stages so the PE never waits on the
            # recip/cast/broadcast latency.
            pv_queue = deque()
            actions = deque()   # (trigger_step, fn)
            gs = 0
            for qi in range(NT):
                ktc = min(WT + 1, NT - qi)
                for kk in range(ktc):
                    while actions and actions[0][0] <= gs:
                        actions.popleft()[1]()
                    pt = attn_step(qi, kk)
                    if len(pv_queue) >= 4:
                        attn_pv(*pv_queue.popleft())
                    pv_queue.append((qi, kk, pt))
                    gs += 1
                actions.append((gs + 1, (lambda q: lambda: attn_out_a(q))(qi)))
                actions.append((gs + 4, (lambda q: lambda: attn_out_b(q))(qi)))
                actions.append((gs + 7, (lambda q: lambda: attn_out_c(q))(qi)))
            while pv_queue:
                attn_pv(*pv_queue.popleft())
            while actions:
                actions.popleft()[1]()

    return nc


def _get_program(T_=T, C_=C, win=WINDOW):
    key = (T_, C_, win)
    if key not in _PROGRAM_CACHE:
        nc = build_program(T_, C_, win)
        nc.finalize()
        _PROGRAM_CACHE[key] = nc
    return _PROGRAM_CACHE[key]


def make_in_maps(x, ve, cos, sin, Wq, Wk, Wv, Wg, Wo):
    """Build the 8 per-core input dicts (host-side sharding/layout prep)."""
    cosT = np.ascontiguousarray(cos[:, 0, :].T).astype(np.float32)  # [64, T]
    sinT = np.ascontiguousarray(sin[:, 0, :].T).astype(np.float32)
    cc = np.concatenate([cosT, cosT], axis=0)            # [128, T]
    ss = np.concatenate([sinT, -sinT], axis=0)           # [128, T]
    # 0/1 multiplicative band masks for the S^T diagonal/far tiles,
    # replicated across the 4 fused q heads
    k_idx = np.arange(P)[:, None]
    q_idx = np.arange(P)[None, :]
    mlo = np.tile((k_idx >= q_idx).astype(np.float32), (1, GH)).astype(F16)
    mhi = np.tile((k_idx < q_idx).astype(np.float32), (1, GH)).astype(F16)

    in_maps = []
    for core in range(N_CORES):
        b, g = divmod(core, N_KV)
        in_maps.append({
            "xT": np.ascontiguousarray(x[b].T).astype(F16),
            "wq": Wq[:, g * GH * HD:(g + 1) * GH * HD].astype(F16),
            "wk": Wk[:, g * HD:(g + 1) * HD].astype(F16),
            "wv": Wv[:, g * HD:(g + 1) * HD].astype(F16),
            "wg": np.ascontiguousarray(Wg[:, g:g + 1]).astype(F16),
            "ve2": (2.0 * ve[b][:, g * HD:(g + 1) * HD]).astype(F16),
            "wo": Wo[g * GH * HD:(g + 1) * GH * HD, :].astype(F16),
            "cc": cc.astype(F16), "ss": ss.astype(F16),
            "mlo": mlo, "mhi": mhi,
        })
    return in_maps


def kernel(x, ve, cos, sin, Wq, Wk, Wv, Wg, Wo, window):
    assert int(window) == WINDOW and x.shape == (B, T, C)
    from concourse.bass_utils import run_bass_kernel_spmd

    nc = _get_program()
    in_maps = make_in_maps(x, ve, cos, sin, Wq, Wk, Wv, Wg, Wo)
    res = run_bass_kernel_spmd(nc, in_maps, core_ids=list(range(N_CORES)))
    out = np.zeros((B, T, C), dtype=np.float32)
    for core in range(N_CORES):
        b = core // N_KV
        out[b] += res.results[core]["out"].astype(np.float32)
    return out


# revision 28
# speedup vs baseline: 1.2443x; 1.0138x over previous
"""Trainium2 Bass kernel for a GQA sliding-window attention layer.

Reference computation (B=2, T=2048, C=2048, 16 Q heads / 4 KV heads, d=128):
    q = x @ Wq; k = x @ Wk; v = x @ Wv (+ sigmoid-gated value embedding)
    q, k = rmsnorm(rope(q)), rmsnorm(rope(k))
    scores masked to the band 0 <= j - i < window (=1024), softmax over j
    out = (p @ v) @ Wo

Sharding: 8 cores = 2 batches x 4 KV groups.  Each core computes its 4 Q
heads / 1 KV head for one batch and a partial output (its 512-row slice of
the Wo contraction); the host sums the 4 partials per batch.

Key implementation notes:
  - fp16 everywhere (same PE/DVE speed as bf16, 8x the mantissa bits).
  - every PE matmul moves fp16 at 1 cycle/row; the only non-GEMM PE work
    is tiny [1,512] row-sums (rms ssq, softmax den) and their fp16
    [128,512] broadcast matmuls (213ns each at full clock).
  - band masks are 0/1 fp16 multiplies on the DVE (4x mode via
    scalar_tensor_tensor) applied to exp(scores) tiles.
  - softmax denominator: fp16 DVE accumulation of the exp tiles, then one
    row-sum matmul per 128-query row.
  - rope reads the projection PSUM directly (the half-swap addressing is
    only legal with a PSUM operand); the swapped-half multiplies run on
    the otherwise-idle Pool engine.
  - gate sigmoid is computed via Exp (1/(1+e^-x)) at the start of the
    attention phase so the ACT engine needs only one table set per phase
    (Sqrt during projection, Exp afterwards).
  - exp has a -2.0 bias folded in (cancels in the softmax ratio) so fp16
    can't overflow even for adversarially aligned q/k.
  - x is DMA'd in token-slice-major order so the first projection wave
    starts after ~1/16 of x has arrived; out is written as fp16.
"""

import numpy as np
from collections import deque

F16 = np.float16

# Problem dims (hardcoded per contest rules)
B, T, C = 2, 2048, 2048
N_HEAD, N_KV, HD, GATE_CH = 16, 4, 128, 32
WINDOW = 1024
P = 128
GH = N_HEAD // N_KV  # q heads per kv head (= per core)
N_CORES = 8

_PROGRAM_CACHE = {}


def build_program(T_=T, C_=C, win=WINDOW, debug=False):
    import concourse.mybir as mybir
    import concourse.tile as tile
    from concourse import bacc

    dt = mybir.dt
    f32 = dt.float32
    f16 = dt.float16
    AF = mybir.ActivationFunctionType
    ALU = mybir.AluOpType

    NT = T_ // P          # token tiles
    KT = C_ // P          # contraction tiles
    WT = win // P         # window tiles
    TS = T_ // 512        # 512-token slices

    nc = bacc.Bacc()

    KTd = C_ // P
    NTd_ = T_ // P
    xT = nc.declare_dram_parameter("xT", [C_, T_], f16, isOutput=False)
    # weights arrive pre-arranged in their SBUF layouts so every DMA moves
    # large contiguous chunks (sub-512B descriptors run at half DMA speed)
    wq = nc.declare_dram_parameter("wq", [P, GH, KTd, HD], f16, isOutput=False)
    wk = nc.declare_dram_parameter("wk", [P, KTd, HD], f16, isOutput=False)
    wv = nc.declare_dram_parameter("wv", [P, KTd, HD], f16, isOutput=False)
    wg = nc.declare_dram_parameter("wg", [GATE_CH, 1], f16, isOutput=False)
    ve2 = nc.declare_dram_parameter("ve2", [P, NTd_, HD], f16, isOutput=False)
    wo = nc.declare_dram_parameter("wo", [P, GH, C_], f16, isOutput=False)
    ccd = nc.declare_dram_parameter("cc", [P, T_], f16, isOutput=False)
    ssd = nc.declare_dram_parameter("ss", [P, T_], f16, isOutput=False)
    mlod = nc.declare_dram_parameter("mlo", [P, GH * P], f16, isOutput=False)
    mhid = nc.declare_dram_parameter("mhi", [P, GH * P], f16, isOutput=False)
    out_d = nc.declare_dram_parameter("out", [T_, C_], f16, isOutput=True)
    if debug:
        NTd = T_ // P
        khat_d = nc.declare_dram_parameter("khat_d", [P, T_], f16, isOutput=True)
        qhat_d = nc.declare_dram_parameter("qhat_d", [P, N_HEAD // N_KV, T_], f16, isOutput=True)
        vsb_d = nc.declare_dram_parameter("vsb_d", [P, NTd, HD], f16, isOutput=True)
        sig_d = nc.declare_dram_parameter("sig_d", [P, NTd], f32, isOutput=True)
        pacc_d = nc.declare_dram_parameter("pacc_d", [P, NTd, 512], f16, isOutput=True)

    with tile.TileContext(nc) as tc:
        with (
            tc.tile_pool(name="singles", bufs=1) as sg,
            tc.tile_pool(name="tails", bufs=3) as tl,
            tc.tile_pool(name="attn", bufs=4) as aw,
            tc.tile_pool(name="outp", bufs=3) as op_pool,
            tc.tile_pool(name="psum", bufs=1, space="PSUM") as pp,
        ):
            # ---- input DMAs --------------------------------------------
            wg_sb = sg.tile([GATE_CH, 1], f16, tag="wg")
            nc.sync.dma_start(out=wg_sb[:], in_=wg[:])
            wq_sb = sg.tile([P, GH, KT, HD], f16, tag="wq")
            wk_sb = sg.tile([P, KT, HD], f16, tag="wk")
            wv_sb = sg.tile([P, KT, HD], f16, tag="wv")
            xt = []
            for kt in range(KT):
                t_ = sg.tile([P, T_], f16, tag=f"xt{kt}", name=f"xt{kt}")
                xt.append(t_)
            # weights on the sync queue (wq split per head so wave A can
            # start early), x halves on the Pool queue in parallel (issue
            # pipelines overlap; transfers share the global DMA engines).
            nc.sync.dma_start(out=wk_sb[:], in_=wk[:])
            nc.sync.dma_start(out=wq_sb[:, 0], in_=wq[:, 0])
            nc.sync.dma_start(out=wq_sb[:, 1], in_=wq[:, 1])
            HT = T_ // 2
            for kt in range(KT):
                nc.gpsimd.dma_start(out=xt[kt][:, 0:HT],
                                    in_=xT[kt * P:(kt + 1) * P, 0:HT])
            nc.sync.dma_start(out=wv_sb[:], in_=wv[:])
            nc.sync.dma_start(out=wq_sb[:, 2], in_=wq[:, 2])
            nc.sync.dma_start(out=wq_sb[:, 3], in_=wq[:, 3])
            # constants needed by the first wave's tails
            cc_sb = sg.tile([P, T_], f16, tag="cc")
            nc.sync.dma_start(out=cc_sb[:], in_=ccd[:])
            ss_sb = sg.tile([P, T_], f16, tag="ss")
            nc.sync.dma_start(out=ss_sb[:], in_=ssd[:])
            ve2_sb = sg.tile([P, NT, HD], f16, tag="ve2")
            nc.sync.dma_start(out=ve2_sb[:], in_=ve2[:])
            mlo_sb = sg.tile([P, GH * P], f16, tag="mlo")
            nc.sync.dma_start(out=mlo_sb[:], in_=mlod[:])
            mhi_sb = sg.tile([P, GH * P], f16, tag="mhi")
            nc.sync.dma_start(out=mhi_sb[:], in_=mhid[:])
            for kt in range(KT):
                nc.gpsimd.dma_start(out=xt[kt][:, HT:T_],
                                    in_=xT[kt * P:(kt + 1) * P, HT:T_])
            wo_sb = sg.tile([P, GH, C_], f16, tag="wo")
            nc.sync.dma_start(out=wo_sb[:], in_=wo[:])

            ones_sb = sg.tile([P, 1], f16, tag="onesb")
            nc.vector.memset(ones_sb[:], 1.0)
            ones1h = sg.tile([1, P], f16, tag="ones1h")
            nc.vector.memset(ones1h[:], 1.0)
            eps_sb = sg.tile([P, 1], f32, tag="epsb")
            nc.vector.memset(eps_sb[:], 1e-6)
            # k gets the 1/sqrt(d) score scale folded into its rms scale:
            # rr_k = (1/sqrt(ssq/HD+eps))/sqrt(HD) = 1/sqrt(ssq + HD*eps)
            epsk_sb = sg.tile([P, 1], f32, tag="epskb")
            nc.vector.memset(epsk_sb[:], HD * 1e-6)
            nexp_b = sg.tile([P, 1], f32, tag="nexpb")
            nc.vector.memset(nexp_b[:], -2.0)

            # persistent intermediates
            qhat = sg.tile([P, GH, T_], f16, tag="qhat")   # normalized roped q, [d, h, t]
            khat = sg.tile([P, T_], f16, tag="khat")       # normalized roped k * isq
            vsb = sg.tile([P, NT, HD], f16, tag="vsb")     # v (gated during attn start)
            sig_sb = sg.tile([P, NT], f32, tag="sigsb")    # gate sigmoids

            # ---- projection phase --------------------------------------
            # gate columns land in SBUF via a per-slice psum bounce (Copy is
            # in every ACT table set, so no table thrash)
            gcol_sb = sg.tile([P, NT], f32, tag="gcolsb")

            def wave_mms(ts, heads, tags):
                sl = slice(ts * 512, ts * 512 + 512)
                items = []
                for head, tag in zip(heads, tags):
                    nb = {"sp": 4, "yps": 2, "ops": 2}[tag]
                    if head == GH + 1:  # V group, [tok, d] per token tile
                        ps = pp.tile([P, 4, HD], f32, tag=tag, bufs=nb,
                                     name=f"psv{ts}")
                    else:
                        ps = pp.tile([P, 512], f32, tag=tag, bufs=nb,
                                     name=f"ps{head}_{ts}")
                    items.append((head, ts, ps))
                for kt in range(KT):
                    for head, _, ps in items:
                        if head == GH + 1:
                            continue
                        if head == 0:
                            nc.tensor.matmul(
                                ps[:], lhsT=wk_sb[:, kt, :],
                                rhs=xt[kt][:, sl],
                                start=(kt == 0), stop=(kt == KT - 1))
                        else:
                            nc.tensor.matmul(
                                ps[:], lhsT=wq_sb[:, head - 1, kt, :],
                                rhs=xt[kt][:, sl],
                                start=(kt == 0), stop=(kt == KT - 1))
                # the V chains share one psum bank, so a chain's start=True
                # would mark the whole bank pending-zero and clobber any
                # other open chain's first term: run them strictly one at a
                # time (j outer, kt inner).
                for head, _, ps in items:
                    if head != GH + 1:
                        continue
                    for j in range(4):
                        tsl = slice(ts * 512 + j * P, ts * 512 + (j + 1) * P)
                        for kt in range(KT):
                            nc.tensor.matmul(
                                ps[:, j, :], lhsT=xt[kt][:, tsl],
                                rhs=wv_sb[:, kt, :],
                                start=(kt == 0), stop=(kt == KT - 1),
                                skip_group_check=True)
                if heads[-1] == GH + 1:
                    gpsl = pp.tile([P, 4], f32, tag="sp", bufs=4,
                                   name=f"gps{ts}")
                    for j in range(4):
                        tsl = slice((ts * 4 + j) * P, (ts * 4 + j + 1) * P)
                        nc.tensor.matmul(
                            gpsl[:, j:j + 1], lhsT=xt[0][0:GATE_CH, tsl],
                            rhs=wg_sb[:], start=True, stop=True,
                            skip_group_check=True)
                    nc.scalar.copy(gcol_sb[:, ts * 4:(ts + 1) * 4], gpsl[:])
                return items

            def tails(items):
                # 1) ropes: free the projection psums first.  The
                #    half-swapped multiplies read the PSUM (partition cross
                #    is only legal there) and run on the Pool engine.
                t1 = []
                for head, ts, ps in items:
                    if head == GH + 1:
                        # raw v copy [tok, d] -> SBUF (gating happens later)
                        nc.scalar.copy(vsb[:, ts * 4:(ts + 1) * 4, :], ps[:])
                        continue
                    sl = slice(ts * 512, ts * 512 + 512)
                    qr = tl.tile([P, 512], f16, tag="qr", bufs=5)
                    nc.vector.tensor_mul(qr[:], ps[:], cc_sb[:, sl])
                    qs = tl.tile([P, 512], f16, tag="qs", bufs=5)
                    nc.vector.tensor_mul(qs[0:64, :], ps[64:128, :], ss_sb[0:64, sl])
                    nc.vector.tensor_mul(qs[64:128, :], ps[0:64, :], ss_sb[64:128, sl])
                    # plain tensor_tensor: the only DVE op class with a
                    # fast mode (2x_1p) for fp16 — STT has none
                    nc.vector.tensor_add(qr[:], qr[:], qs[:])
                    t1.append((head, sl, qr))
                # 2) squares on Pool
                q2s = []
                for head, sl, qr in t1:
                    q2 = tl.tile([P, 512], f16, tag="q2", bufs=5)
                    nc.gpsimd.tensor_mul(q2[:], qr[:], qr[:])
                    q2s.append(q2)
                # 3) per group: row-sum matmul, sqrt, recip, cast, broadcast
                #    matmul, final scale.  sp-tag psums rotate: each is freed
                #    by the fast ACT/DVE op right behind it.
                for (head, sl, qr), q2 in zip(t1, q2s):
                    ssq = pp.tile([1, 512], f32, tag="sp", bufs=4, name="ssq")
                    nc.tensor.matmul(ssq[:], lhsT=ones_sb[:], rhs=q2[:],
                                     start=True, stop=True)
                    sq = tl.tile([1, 512], f32, tag="sq", bufs=4)
                    if head == 0:
                        nc.scalar.activation(sq[:], ssq[:], AF.Sqrt,
                                             bias=epsk_sb[0:1, :], scale=1.0)
                    else:
                        nc.scalar.activation(sq[:], ssq[:], AF.Sqrt,
                                             bias=eps_sb[0:1, :], scale=1.0 / HD)
                    rr = tl.tile([1, 512], f32, tag="rr", bufs=4)
                    nc.vector.reciprocal_approx_fast(rr[:], sq[:])
                    rr16 = tl.tile([1, 512], f16, tag="rr16", bufs=4)
                    nc.scalar.copy(rr16[:], rr[:])
                    rrb = pp.tile([P, 512], f32, tag="sp", bufs=4, name="rrb")
                    nc.tensor.matmul(rrb[:], lhsT=ones1h[:], rhs=rr16[:],
                                     start=True, stop=True)
                    dest = khat[:, sl] if head == 0 else qhat[:, head - 1, sl]
                    nc.vector.tensor_mul(dest, qr[:], rrb[:])

            for ts in range(TS):
                items_a = wave_mms(ts, [0, 1, 2], ["sp", "sp", "sp"])
                items_b = wave_mms(ts, [3, 4, GH + 1], ["yps", "yps", "ops"])
                tails(items_a)
                tails(items_b)

            # ---- gate sigmoids + v gating (one Exp table load) ---------
            egate = tl.tile([P, NT], f32, tag="egate", bufs=1)
            nc.scalar.activation(egate[:], gcol_sb[:], AF.Exp, scale=-1.0)  # e^-g
            ep1 = tl.tile([P, NT], f32, tag="ep1", bufs=1)
            nc.vector.tensor_scalar_add(ep1[:], egate[:], 1.0)
            nc.vector.reciprocal_approx_fast(sig_sb[:], ep1[:])         # sigmoid
            for tt in range(NT):
                # v += sigmoid * ve2 (ve2 pre-scaled by 2), in place
                nc.vector.scalar_tensor_tensor(
                    out=vsb[:, tt, :], in0=ve2_sb[:, tt, :],
                    scalar=sig_sb[:, tt:tt + 1],
                    in1=vsb[:, tt, :], op0=ALU.mult, op1=ALU.add)

            # ---- attention + out-projection ----------------------------
            CO = C_ // 512  # output column chunks

            yps_of = {}
            pacc_of = {}

            def attn_step(qi, kk):
                ktc = min(WT + 1, NT - qi)
                kt = qi + kk
                qs4 = qhat[:, :, qi * P:(qi + 1) * P]   # [d, (h, q)] = 512 wide
                sp = pp.tile([P, 512], f32, tag="sp", bufs=4, name=f"sp{qi}_{kk}")
                nc.tensor.matmul(sp[:], lhsT=khat[:, kt * P:(kt + 1) * P],
                                 rhs=qs4, start=True, stop=True)
                pt = aw.tile([P, GH * P], f16, tag="ptq", bufs=6)
                nc.scalar.activation(pt[:], sp[:], AF.Exp, bias=nexp_b[:])
                masked = (kk == 0) or (kk == WT and ktc == WT + 1)
                if masked:
                    msk = mlo_sb if kk == 0 else mhi_sb
                    nc.vector.tensor_mul(pt[:], pt[:], msk[:])
                # denominator accumulation (fp16, 4x TSP mode)
                if kk == 0:
                    pacc = aw.tile([P, 512], f16, tag="pacc", bufs=3,
                                   name=f"pacc{qi}")
                    pacc_of[qi] = pacc
                    # init as a mul: pt is already masked and the mask is
                    # 0/1, so multiplying by it again is a no-op copy
                    nc.vector.tensor_mul(pacc[:], pt[:], mlo_sb[:])
                else:
                    pacc = pacc_of[qi]
                    nc.vector.tensor_add(pacc[:], pacc[:], pt[:])
                return pt

            def attn_pv(qi, kk, pt):
                ktc = min(WT + 1, NT - qi)
                kt = qi + kk
                if kk == 0:
                    yps_of[qi] = pp.tile([P, GH * P], f32, tag="yps", bufs=2,
                                         name=f"yp{qi}")
                nc.tensor.matmul(yps_of[qi][:], lhsT=vsb[:, kt, :], rhs=pt[:],
                                 start=(kk == 0), stop=(kk == ktc - 1))

            rd16_of = {}
            yq_of = {}

            def attn_out_a(qi):
                # denominator row-sum -> reciprocal -> fp16 cast
                if debug:
                    nc.sync.dma_start(out=pacc_d[:, qi, :], in_=pacc_of[qi][:])
                denp = pp.tile([1, 512], f32, tag="sp", bufs=4, name=f"den{qi}")
                nc.tensor.matmul(denp[:], lhsT=ones_sb[:], rhs=pacc_of[qi][:],
                                 start=True, stop=True)
                rd = tl.tile([1, 512], f32, tag="rd", bufs=2)
                nc.vector.reciprocal_approx_fast(rd[:], denp[:])
                rd16 = tl.tile([1, 512], f16, tag="rd16", bufs=2)
                nc.scalar.copy(rd16[:], rd[:])
                rd16_of[qi] = rd16

            def attn_out_b(qi):
                # broadcast + normalize straight from the two psums
                rdb = pp.tile([P, 512], f32, tag="sp", bufs=4, name=f"rdb{qi}")
                nc.tensor.matmul(rdb[:], lhsT=ones1h[:], rhs=rd16_of[qi][:],
                                 start=True, stop=True)
                yut = op_pool.tile([P, GH * P], f16, tag="yut", bufs=2)
                nc.vector.tensor_copy(yut[:], yps_of[qi][:])   # frees yps
                yq = op_pool.tile([P, GH * P], f16, tag="yq", bufs=2)
                nc.vector.tensor_mul(yq[:], yut[:], rdb[:])
                yq_of[qi] = yq

            def attn_out_c(qi):
                qsl = slice(qi * P, (qi + 1) * P)
                yq = yq_of[qi]
                ob = op_pool.tile([P, C_], f16, tag="ob", bufs=2)
                for co in range(CO):
                    osl = slice(co * 512, co * 512 + 512)
                    ops = pp.tile([P, 512], f32, tag="ops", bufs=2,
                                  name=f"ops{qi}_{co}")
                    for h in range(GH):
                        nc.tensor.matmul(
                            ops[:], lhsT=yq[:, h * P:(h + 1) * P],
                            rhs=wo_sb[:, h, osl],
                            start=(h == 0), stop=(h == GH - 1))
                    if co % 2 == 0:
                        nc.scalar.copy(ob[:, osl], ops[:])
                    else:
                        nc.vector.tensor_copy(out=ob[:, osl], in_=ops[:])
                nc.sync.dma_start(out=out_d[qsl, :], in_=ob[:])

            if debug:
                nc.sync.dma_start(out=khat_d[:], in_=khat[:])
                nc.sync.dma_start(out=qhat_d[:], in_=qhat[:])
                nc.sync.dma_start(out=vsb_d[:], in_=vsb[:])
                nc.sync.dma_start(out=sig_d[:], in_=sig_sb[:])

            # software pipeline over global attention steps: PV trails the
            # scores/exp stream by 4 steps; each finished row's out chain is
            # emitted in three staggered stages so the PE never waits on the
            # recip/cast/broadcast latency.
            pv_queue = deque()
            actions = deque()   # (trigger_step, fn)
            gs = 0
            for qi in range(NT):
                ktc = min(WT + 1, NT - qi)
                for kk in range(ktc):
                    while actions and actions[0][0] <= gs:
                        actions.popleft()[1]()
                    pt = attn_step(qi, kk)
                    if len(pv_queue) >= 4:
                        attn_pv(*pv_queue.popleft())
                    pv_queue.append((qi, kk, pt))
                    gs += 1
                actions.append((gs + 1, (lambda q: lambda: attn_out_a(q))(qi)))
                actions.append((gs + 4, (lambda q: lambda: attn_out_b(q))(qi)))
                actions.append((gs + 7, (lambda q: lambda: attn_out_c(q))(qi)))
            while pv_queue:
                attn_pv(*pv_queue.popleft())
            while actions:
                actions.popleft()[1]()

    return nc


def _get_program(T_=T, C_=C, win=WINDOW):
    key = (T_, C_, win)
    if key not in _PROGRAM_CACHE:
        nc = build_program(T_, C_, win)
        nc.finalize()
        _PROGRAM_CACHE[key] = nc
    return _PROGRAM_CACHE[key]


def make_in_maps(x, ve, cos, sin, Wq, Wk, Wv, Wg, Wo):
    """Build the 8 per-core input dicts (host-side sharding/layout prep)."""
    cosT = np.ascontiguousarray(cos[:, 0, :].T).astype(np.float32)  # [64, T]
    sinT = np.ascontiguousarray(sin[:, 0, :].T).astype(np.float32)
    cc = np.concatenate([cosT, cosT], axis=0)            # [128, T]
    ss = np.concatenate([sinT, -sinT], axis=0)           # [128, T]
    # 0/1 multiplicative band masks for the S^T diagonal/far tiles,
    # replicated across the 4 fused q heads
    k_idx = np.arange(P)[:, None]
    q_idx = np.arange(P)[None, :]
    mlo = np.tile((k_idx >= q_idx).astype(np.float32), (1, GH)).astype(F16)
    mhi = np.tile((k_idx < q_idx).astype(np.float32), (1, GH)).astype(F16)

    KT, NT = C // P, T // P
    in_maps = []
    for core in range(N_CORES):
        b, g = divmod(core, N_KV)
        wq_l = Wq[:, g * GH * HD:(g + 1) * GH * HD]
        wk_l = Wk[:, g * HD:(g + 1) * HD]
        wv_l = Wv[:, g * HD:(g + 1) * HD]
        wo_l = Wo[g * GH * HD:(g + 1) * GH * HD, :]
        ve_l = 2.0 * ve[b][:, g * HD:(g + 1) * HD]
        in_maps.append({
            "xT": np.ascontiguousarray(x[b].T).astype(F16),
            # pre-arranged in SBUF layout for contiguous DMA
            "wq": np.ascontiguousarray(
                wq_l.reshape(KT, P, GH, HD).transpose(1, 2, 0, 3)).astype(F16),
            "wk": np.ascontiguousarray(
                wk_l.reshape(KT, P, HD).transpose(1, 0, 2)).astype(F16),
            "wv": np.ascontiguousarray(
                wv_l.reshape(KT, P, HD).transpose(1, 0, 2)).astype(F16),
            "wg": np.ascontiguousarray(Wg[:, g:g + 1]).astype(F16),
            "ve2": np.ascontiguousarray(
                ve_l.reshape(NT, P, HD).transpose(1, 0, 2)).astype(F16),
            "wo": np.ascontiguousarray(
                wo_l.reshape(GH, P, C).transpose(1, 0, 2)).astype(F16),
            "cc": cc.astype(F16), "ss": ss.astype(F16),
            "mlo": mlo, "mhi": mhi,
        })
    return in_maps


def kernel(x, ve, cos, sin, Wq, Wk, Wv, Wg, Wo, window):
    assert int(window) == WINDOW and x.shape == (B, T, C)
    from concourse.bass_utils import run_bass_kernel_spmd

    nc = _get_program()
    in_maps = make_in_maps(x, ve, cos, sin, Wq, Wk, Wv, Wg, Wo)
    res = run_bass_kernel_spmd(nc, in_maps, core_ids=list(range(N_CORES)))
    out = np.zeros((B, T, C), dtype=np.float32)
    for core in range(N_CORES):
        b = core // N_KV
        out[b] += res.results[core]["out"].astype(np.float32)
    return out


# revision 30
# speedup vs baseline: 1.2667x; 1.0180x over previous
"""Trainium2 Bass kernel for a GQA sliding-window attention layer.

Reference computation (B=2, T=2048, C=2048, 16 Q heads / 4 KV heads, d=128):
    q = x @ Wq; k = x @ Wk; v = x @ Wv (+ sigmoid-gated value embedding)
    q, k = rmsnorm(rope(q)), rmsnorm(rope(k))
    scores masked to the band 0 <= j - i < window (=1024), softmax over j
    out = (p @ v) @ Wo

Sharding: 8 cores = 2 batches x 4 KV groups.  Each core computes its 4 Q
heads / 1 KV head for one batch and a partial output (its 512-row slice of
the Wo contraction); the host sums the 4 partials per batch.

Key implementation notes:
  - fp16 everywhere (same PE/DVE speed as bf16, 8x the mantissa bits).
  - every PE matmul moves fp16 at 1 cycle/row; the only non-GEMM PE work
    is tiny [1,512] row-sums (rms ssq, softmax den) and their fp16
    [128,512] broadcast matmuls (213ns each at full clock).
  - band masks are 0/1 fp16 multiplies on the DVE (4x mode via
    scalar_tensor_tensor) applied to exp(scores) tiles.
  - softmax denominator: fp16 DVE accumulation of the exp tiles, then one
    row-sum matmul per 128-query row.
  - rope reads the projection PSUM directly (the half-swap addressing is
    only legal with a PSUM operand); the swapped-half multiplies run on
    the otherwise-idle Pool engine.
  - gate sigmoid is computed via Exp (1/(1+e^-x)) at the start of the
    attention phase so the ACT engine needs only one table set per phase
    (Sqrt during projection, Exp afterwards).
  - exp has a -2.0 bias folded in (cancels in the softmax ratio) so fp16
    can't overflow even for adversarially aligned q/k.
  - x is DMA'd in token-slice-major order so the first projection wave
    starts after ~1/16 of x has arrived; out is written as fp16.
"""

import numpy as np
from collections import deque

F16 = np.float16

# Problem dims (hardcoded per contest rules)
B, T, C = 2, 2048, 2048
N_HEAD, N_KV, HD, GATE_CH = 16, 4, 128, 32
WINDOW = 1024
P = 128
GH = N_HEAD // N_KV  # q heads per kv head (= per core)
N_CORES = 8

_PROGRAM_CACHE = {}


def build_program(T_=T, C_=C, win=WINDOW, debug=False):
    import concourse.mybir as mybir
    import concourse.tile as tile
    from concourse import bacc

    dt = mybir.dt
    f32 = dt.float32
    f16 = dt.float16
    AF = mybir.ActivationFunctionType
    ALU = mybir.AluOpType

    NT = T_ // P          # token tiles
    KT = C_ // P          # contraction tiles
    WT = win // P         # window tiles
    TS = T_ // 512        # 512-token slices

    nc = bacc.Bacc()

    KTd = C_ // P
    NTd_ = T_ // P
    xT = nc.declare_dram_parameter("xT", [C_, T_], f16, isOutput=False)
    # weights arrive pre-arranged in their SBUF layouts so every DMA moves
    # large contiguous chunks (sub-512B descriptors run at half DMA speed)
    wq = nc.declare_dram_parameter("wq", [P, GH, KTd, HD], f16, isOutput=False)
    wk = nc.declare_dram_parameter("wk", [P, KTd, HD], f16, isOutput=False)
    wv = nc.declare_dram_parameter("wv", [P, KTd, HD], f16, isOutput=False)
    wg = nc.declare_dram_parameter("wg", [GATE_CH, 1], f16, isOutput=False)
    ve2 = nc.declare_dram_parameter("ve2", [P, NTd_, HD], f16, isOutput=False)
    wo = nc.declare_dram_parameter("wo", [P, GH, C_], f16, isOutput=False)
    ccd = nc.declare_dram_parameter("cc", [P, T_], f16, isOutput=False)
    ssd = nc.declare_dram_parameter("ss", [P, T_], f16, isOutput=False)
    mlod = nc.declare_dram_parameter("mlo", [P, GH * P], f16, isOutput=False)
    mhid = nc.declare_dram_parameter("mhi", [P, GH * P], f16, isOutput=False)
    out_d = nc.declare_dram_parameter("out", [T_, C_], f16, isOutput=True)
    if debug:
        NTd = T_ // P
        khat_d = nc.declare_dram_parameter("khat_d", [P, T_], f16, isOutput=True)
        qhat_d = nc.declare_dram_parameter("qhat_d", [P, N_HEAD // N_KV, T_], f16, isOutput=True)
        vsb_d = nc.declare_dram_parameter("vsb_d", [P, NTd, HD], f16, isOutput=True)
        sig_d = nc.declare_dram_parameter("sig_d", [P, NTd], f32, isOutput=True)
        pacc_d = nc.declare_dram_parameter("pacc_d", [P, NTd, 512], f16, isOutput=True)

    with tile.TileContext(nc) as tc:
        with (
            tc.tile_pool(name="singles", bufs=1) as sg,
            tc.tile_pool(name="tails", bufs=3) as tl,
            tc.tile_pool(name="attn", bufs=4) as aw,
            tc.tile_pool(name="outp", bufs=3) as op_pool,
            tc.tile_pool(name="psum", bufs=1, space="PSUM") as pp,
        ):
            # ---- input DMAs --------------------------------------------
            wg_sb = sg.tile([GATE_CH, 1], f16, tag="wg")
            nc.sync.dma_start(out=wg_sb[:], in_=wg[:])
            wq_sb = sg.tile([P, GH, KT, HD], f16, tag="wq")
            wk_sb = sg.tile([P, KT, HD], f16, tag="wk")
            wv_sb = sg.tile([P, KT, HD], f16, tag="wv")
            xt = []
            for kt in range(KT):
                t_ = sg.tile([P, T_], f16, tag=f"xt{kt}", name=f"xt{kt}")
                xt.append(t_)
            # weights on the sync queue (wq split per head so wave A can
            # start early), x halves on the Pool queue in parallel (issue
            # pipelines overlap; transfers share the global DMA engines).
            nc.sync.dma_start(out=wk_sb[:], in_=wk[:])
            nc.sync.dma_start(out=wq_sb[:, 0], in_=wq[:, 0])
            nc.sync.dma_start(out=wq_sb[:, 1], in_=wq[:, 1])
            HT = T_ // 2
            for kt in range(KT):
                nc.gpsimd.dma_start(out=xt[kt][:, 0:HT],
                                    in_=xT[kt * P:(kt + 1) * P, 0:HT])
            nc.sync.dma_start(out=wv_sb[:], in_=wv[:])
            nc.sync.dma_start(out=wq_sb[:, 2], in_=wq[:, 2])
            nc.sync.dma_start(out=wq_sb[:, 3], in_=wq[:, 3])
            # constants needed by the first wave's tails
            cc_sb = sg.tile([P, T_], f16, tag="cc")
            nc.sync.dma_start(out=cc_sb[:], in_=ccd[:])
            ss_sb = sg.tile([P, T_], f16, tag="ss")
            nc.sync.dma_start(out=ss_sb[:], in_=ssd[:])
            ve2_sb = sg.tile([P, NT, HD], f16, tag="ve2")
            nc.sync.dma_start(out=ve2_sb[:], in_=ve2[:])
            mlo_sb = sg.tile([P, GH * P], f16, tag="mlo")
            nc.sync.dma_start(out=mlo_sb[:], in_=mlod[:])
            mhi_sb = sg.tile([P, GH * P], f16, tag="mhi")
            nc.sync.dma_start(out=mhi_sb[:], in_=mhid[:])
            for kt in range(KT):
                nc.gpsimd.dma_start(out=xt[kt][:, HT:T_],
                                    in_=xT[kt * P:(kt + 1) * P, HT:T_])
            wo_sb = sg.tile([P, GH, C_], f16, tag="wo")
            nc.sync.dma_start(out=wo_sb[:], in_=wo[:])

            ones_sb = sg.tile([P, 1], f16, tag="onesb")
            nc.vector.memset(ones_sb[:], 1.0)
            ones1h = sg.tile([1, P], f16, tag="ones1h")
            nc.vector.memset(ones1h[:], 1.0)
            eps_sb = sg.tile([P, 1], f32, tag="epsb")
            nc.vector.memset(eps_sb[:], 1e-6)
            # k gets the 1/sqrt(d) score scale folded into its rms scale:
            # rr_k = (1/sqrt(ssq/HD+eps))/sqrt(HD) = 1/sqrt(ssq + HD*eps)
            epsk_sb = sg.tile([P, 1], f32, tag="epskb")
            nc.vector.memset(epsk_sb[:], HD * 1e-6)
            nexp_b = sg.tile([P, 1], f32, tag="nexpb")
            nc.vector.memset(nexp_b[:], -2.0)

            # persistent intermediates
            qhat = sg.tile([P, GH, T_], f16, tag="qhat")   # normalized roped q, [d, h, t]
            khat = sg.tile([P, T_], f16, tag="khat")       # normalized roped k * isq
            vsb = sg.tile([P, NT, HD], f16, tag="vsb")     # v (gated during attn start)
            sig_sb = sg.tile([P, NT], f32, tag="sigsb")    # gate sigmoids

            # ---- projection phase --------------------------------------
            # gate columns land in SBUF via a per-slice psum bounce (Copy is
            # in every ACT table set, so no table thrash)
            gcol_sb = sg.tile([P, NT], f32, tag="gcolsb")

            def wave_mms(ts, heads, tags):
                sl = slice(ts * 512, ts * 512 + 512)
                items = []
                for head, tag in zip(heads, tags):
                    nb = {"sp": 4, "yps": 2, "ops": 2}[tag]
                    if head == GH + 1:  # V group, [tok, d] per token tile
                        ps = pp.tile([P, 4, HD], f32, tag=tag, bufs=nb,
                                     name=f"psv{ts}")
                    else:
                        ps = pp.tile([P, 512], f32, tag=tag, bufs=nb,
                                     name=f"ps{head}_{ts}")
                    items.append((head, ts, ps))
                for kt in range(KT):
                    for head, _, ps in items:
                        if head == GH + 1:
                            continue
                        if head == 0:
                            nc.tensor.matmul(
                                ps[:], lhsT=wk_sb[:, kt, :],
                                rhs=xt[kt][:, sl],
                                start=(kt == 0), stop=(kt == KT - 1))
                        else:
                            nc.tensor.matmul(
                                ps[:], lhsT=wq_sb[:, head - 1, kt, :],
                                rhs=xt[kt][:, sl],
                                start=(kt == 0), stop=(kt == KT - 1))
                # the V chains share one psum bank, so a chain's start=True
                # would mark the whole bank pending-zero and clobber any
                # other open chain's first term: run them strictly one at a
                # time (j outer, kt inner).
                for head, _, ps in items:
                    if head != GH + 1:
                        continue
                    for j in range(4):
                        tsl = slice(ts * 512 + j * P, ts * 512 + (j + 1) * P)
                        for kt in range(KT):
                            nc.tensor.matmul(
                                ps[:, j, :], lhsT=xt[kt][:, tsl],
                                rhs=wv_sb[:, kt, :],
                                start=(kt == 0), stop=(kt == KT - 1),
                                skip_group_check=True)
                if heads[-1] == GH + 1:
                    gpsl = pp.tile([P, 4], f32, tag="sp", bufs=4,
                                   name=f"gps{ts}")
                    for j in range(4):
                        tsl = slice((ts * 4 + j) * P, (ts * 4 + j + 1) * P)
                        nc.tensor.matmul(
                            gpsl[:, j:j + 1], lhsT=xt[0][0:GATE_CH, tsl],
                            rhs=wg_sb[:], start=True, stop=True,
                            skip_group_check=True)
                    nc.scalar.copy(gcol_sb[:, ts * 4:(ts + 1) * 4], gpsl[:])
                return items

            def tails_pre(items):
                # ropes + squares: frees the projection psums quickly and
                # starts the per-group chains on ACT/DVE/Pool
                t1 = []
                for head, ts, ps in items:
                    if head == GH + 1:
                        # raw v copy [tok, d] -> SBUF (gating happens later)
                        nc.scalar.copy(vsb[:, ts * 4:(ts + 1) * 4, :], ps[:])
                        continue
                    sl = slice(ts * 512, ts * 512 + 512)
                    qp = tl.tile([P, 512], f16, tag="qp", bufs=3)
                    nc.scalar.copy(qp[:], ps[:])
                    qs = tl.tile([P, 512], f16, tag="qs", bufs=3)
                    nc.vector.tensor_mul(qs[0:64, :], ps[64:128, :], ss_sb[0:64, sl])
                    nc.vector.tensor_mul(qs[64:128, :], ps[0:64, :], ss_sb[64:128, sl])
                    qr = tl.tile([P, 512], f16, tag="qr", bufs=5)
                    nc.vector.tensor_mul(qr[:], qp[:], cc_sb[:, sl])
                    nc.vector.tensor_add(qr[:], qr[:], qs[:])
                    q2 = tl.tile([P, 512], f16, tag="q2", bufs=5)
                    nc.gpsimd.tensor_mul(q2[:], qr[:], qr[:])
                    t1.append((head, sl, qr, q2))
                return t1

            def tails_post(t1):
                # row-sum matmul, sqrt, recip, cast, broadcast matmul, final
                # scale.  Emitted one wave later so the PE only reaches these
                # matmuls when their chains have already drained.
                for head, sl, qr, q2 in t1:
                    ssq = pp.tile([1, 512], f32, tag="sp", bufs=4, name="ssq")
                    nc.tensor.matmul(ssq[:], lhsT=ones_sb[:], rhs=q2[:],
                                     start=True, stop=True)
                    sq = tl.tile([1, 512], f32, tag="sq", bufs=3)
                    if head == 0:
                        nc.scalar.activation(sq[:], ssq[:], AF.Sqrt,
                                             bias=epsk_sb[0:1, :], scale=1.0)
                    else:
                        nc.scalar.activation(sq[:], ssq[:], AF.Sqrt,
                                             bias=eps_sb[0:1, :], scale=1.0 / HD)
                    rr = tl.tile([1, 512], f32, tag="rr", bufs=3)
                    nc.vector.reciprocal_approx_fast(rr[:], sq[:])
                    rr16 = tl.tile([1, 512], f16, tag="rr16", bufs=4)
                    nc.scalar.copy(rr16[:], rr[:])
                    rrb = pp.tile([P, 512], f32, tag="sp", bufs=4, name="rrb")
                    nc.tensor.matmul(rrb[:], lhsT=ones1h[:], rhs=rr16[:],
                                     start=True, stop=True)
                    dest = khat[:, sl] if head == 0 else qhat[:, head - 1, sl]
                    nc.vector.tensor_mul(dest, qr[:], rrb[:])

            pending = []
            for ts in range(TS):
                items_a = wave_mms(ts, [0, 1, 2], ["sp", "sp", "sp"])
                items_b = wave_mms(ts, [3, 4, GH + 1], ["yps", "yps", "ops"])
                if pending:
                    tails_post(pending)
                pending = tails_pre(items_a) + tails_pre(items_b)
            tails_post(pending)

            # ---- gate sigmoids + v gating (one Exp table load) ---------
            egate = tl.tile([P, NT], f32, tag="egate", bufs=1)
            nc.scalar.activation(egate[:], gcol_sb[:], AF.Exp, scale=-1.0)  # e^-g
            ep1 = tl.tile([P, NT], f32, tag="ep1", bufs=1)
            nc.vector.tensor_scalar_add(ep1[:], egate[:], 1.0)
            nc.vector.reciprocal_approx_fast(sig_sb[:], ep1[:])         # sigmoid
            for tt in range(NT):
                # v += sigmoid * ve2 (ve2 pre-scaled by 2), in place
                nc.vector.scalar_tensor_tensor(
                    out=vsb[:, tt, :], in0=ve2_sb[:, tt, :],
                    scalar=sig_sb[:, tt:tt + 1],
                    in1=vsb[:, tt, :], op0=ALU.mult, op1=ALU.add)

            # ---- attention + out-projection ----------------------------
            CO = C_ // 512  # output column chunks

            yps_of = {}
            pacc_of = {}

            def attn_step(qi, kk):
                ktc = min(WT + 1, NT - qi)
                kt = qi + kk
                qs4 = qhat[:, :, qi * P:(qi + 1) * P]   # [d, (h, q)] = 512 wide
                sp = pp.tile([P, 512], f32, tag="sp", bufs=4, name=f"sp{qi}_{kk}")
                nc.tensor.matmul(sp[:], lhsT=khat[:, kt * P:(kt + 1) * P],
                                 rhs=qs4, start=True, stop=True)
                pt = aw.tile([P, GH * P], f16, tag="ptq", bufs=6)
                nc.scalar.activation(pt[:], sp[:], AF.Exp, bias=nexp_b[:])
                masked = (kk == 0) or (kk == WT and ktc == WT + 1)
                if masked:
                    msk = mlo_sb if kk == 0 else mhi_sb
                    nc.vector.tensor_mul(pt[:], pt[:], msk[:])
                # denominator accumulation (fp16, 4x TSP mode)
                if kk == 0:
                    pacc = aw.tile([P, 512], f16, tag="pacc", bufs=3,
                                   name=f"pacc{qi}")
                    pacc_of[qi] = pacc
                    # init as a mul: pt is already masked and the mask is
                    # 0/1, so multiplying by it again is a no-op copy
                    nc.vector.tensor_mul(pacc[:], pt[:], mlo_sb[:])
                else:
                    pacc = pacc_of[qi]
                    nc.vector.tensor_add(pacc[:], pacc[:], pt[:])
                return pt

            def attn_pv(qi, kk, pt):
                ktc = min(WT + 1, NT - qi)
                kt = qi + kk
                if kk == 0:
                    yps_of[qi] = pp.tile([P, GH * P], f32, tag="yps", bufs=2,
                                         name=f"yp{qi}")
                nc.tensor.matmul(yps_of[qi][:], lhsT=vsb[:, kt, :], rhs=pt[:],
                                 start=(kk == 0), stop=(kk == ktc - 1))

            rd16_of = {}
            yq_of = {}

            def attn_out_a(qi):
                # denominator row-sum -> reciprocal -> fp16 cast
                if debug:
                    nc.sync.dma_start(out=pacc_d[:, qi, :], in_=pacc_of[qi][:])
                denp = pp.tile([1, 512], f32, tag="sp", bufs=4, name=f"den{qi}")
                nc.tensor.matmul(denp[:], lhsT=ones_sb[:], rhs=pacc_of[qi][:],
                                 start=True, stop=True)
                rd = tl.tile([1, 512], f32, tag="rd", bufs=2)
                nc.vector.reciprocal_approx_fast(rd[:], denp[:])
                rd16 = tl.tile([1, 512], f16, tag="rd16", bufs=2)
                nc.scalar.copy(rd16[:], rd[:])
                rd16_of[qi] = rd16

            def attn_out_b(qi):
                # broadcast + normalize straight from the two psums
                rdb = pp.tile([P, 512], f32, tag="sp", bufs=4, name=f"rdb{qi}")
                nc.tensor.matmul(rdb[:], lhsT=ones1h[:], rhs=rd16_of[qi][:],
                                 start=True, stop=True)
                yut = op_pool.tile([P, GH * P], f16, tag="yut", bufs=2)
                nc.vector.tensor_copy(yut[:], yps_of[qi][:])   # frees yps
                yq = op_pool.tile([P, GH * P], f16, tag="yq", bufs=2)
                nc.vector.tensor_mul(yq[:], yut[:], rdb[:])
                yq_of[qi] = yq

            def attn_out_c(qi):
                qsl = slice(qi * P, (qi + 1) * P)
                yq = yq_of[qi]
                ob = op_pool.tile([P, C_], f16, tag="ob", bufs=2)
                for co in range(CO):
                    osl = slice(co * 512, co * 512 + 512)
                    ops = pp.tile([P, 512], f32, tag="ops", bufs=2,
                                  name=f"ops{qi}_{co}")
                    for h in range(GH):
                        nc.tensor.matmul(
                            ops[:], lhsT=yq[:, h * P:(h + 1) * P],
                            rhs=wo_sb[:, h, osl],
                            start=(h == 0), stop=(h == GH - 1))
                    if co % 2 == 0:
                        nc.scalar.copy(ob[:, osl], ops[:])
                    else:
                        nc.vector.tensor_copy(out=ob[:, osl], in_=ops[:])
                nc.sync.dma_start(out=out_d[qsl, :], in_=ob[:])

            if debug:
                nc.sync.dma_start(out=khat_d[:], in_=khat[:])
                nc.sync.dma_start(out=qhat_d[:], in_=qhat[:])
                nc.sync.dma_start(out=vsb_d[:], in_=vsb[:])
                nc.sync.dma_start(out=sig_d[:], in_=sig_sb[:])

            # software pipeline over global attention steps: PV trails the
            # scores/exp stream by 4 steps; each finished row's out chain is
            # emitted in three staggered stages so the PE never waits on the
            # recip/cast/broadcast latency.
            pv_queue = deque()
            actions = deque()   # (trigger_step, fn)
            gs = 0
            for qi in range(NT):
                ktc = min(WT + 1, NT - qi)
                for kk in range(ktc):
                    while actions and actions[0][0] <= gs:
                        actions.popleft()[1]()
                    pt = attn_step(qi, kk)
                    if len(pv_queue) >= 4:
                        attn_pv(*pv_queue.popleft())
                    pv_queue.append((qi, kk, pt))
                    gs += 1
                actions.append((gs + 1, (lambda q: lambda: attn_out_a(q))(qi)))
                actions.append((gs + 4, (lambda q: lambda: attn_out_b(q))(qi)))
                actions.append((gs + 7, (lambda q: lambda: attn_out_c(q))(qi)))
            while pv_queue:
                attn_pv(*pv_queue.popleft())
            while actions:
                actions.popleft()[1]()

    return nc


def _get_program(T_=T, C_=C, win=WINDOW):
    key = (T_, C_, win)
    if key not in _PROGRAM_CACHE:
        nc = build_program(T_, C_, win)
        nc.finalize()
        _PROGRAM_CACHE[key] = nc
    return _PROGRAM_CACHE[key]


def make_in_maps(x, ve, cos, sin, Wq, Wk, Wv, Wg, Wo):
    """Build the 8 per-core input dicts (host-side sharding/layout prep)."""
    cosT = np.ascontiguousarray(cos[:, 0, :].T).astype(np.float32)  # [64, T]
    sinT = np.ascontiguousarray(sin[:, 0, :].T).astype(np.float32)
    cc = np.concatenate([cosT, cosT], axis=0)            # [128, T]
    ss = np.concatenate([sinT, -sinT], axis=0)           # [128, T]
    # 0/1 multiplicative band masks for the S^T diagonal/far tiles,
    # replicated across the 4 fused q heads
    k_idx = np.arange(P)[:, None]
    q_idx = np.arange(P)[None, :]
    mlo = np.tile((k_idx >= q_idx).astype(np.float32), (1, GH)).astype(F16)
    mhi = np.tile((k_idx < q_idx).astype(np.float32), (1, GH)).astype(F16)

    KT, NT = C // P, T // P
    in_maps = []
    for core in range(N_CORES):
        b, g = divmod(core, N_KV)
        wq_l = Wq[:, g * GH * HD:(g + 1) * GH * HD]
        wk_l = Wk[:, g * HD:(g + 1) * HD]
        wv_l = Wv[:, g * HD:(g + 1) * HD]
        wo_l = Wo[g * GH * HD:(g + 1) * GH * HD, :]
        ve_l = 2.0 * ve[b][:, g * HD:(g + 1) * HD]
        in_maps.append({
            "xT": np.ascontiguousarray(x[b].T).astype(F16),
            # pre-arranged in SBUF layout for contiguous DMA
            "wq": np.ascontiguousarray(
                wq_l.reshape(KT, P, GH, HD).transpose(1, 2, 0, 3)).astype(F16),
            "wk": np.ascontiguousarray(
                wk_l.reshape(KT, P, HD).transpose(1, 0, 2)).astype(F16),
            "wv": np.ascontiguousarray(
                wv_l.reshape(KT, P, HD).transpose(1, 0, 2)).astype(F16),
            "wg": np.ascontiguousarray(Wg[:, g:g + 1]).astype(F16),
            "ve2": np.ascontiguousarray(
                ve_l.reshape(NT, P, HD).transpose(1, 0, 2)).astype(F16),
            "wo": np.ascontiguousarray(
                wo_l.reshape(GH, P, C).transpose(1, 0, 2)).astype(F16),
            "cc": cc.astype(F16), "ss": ss.astype(F16),
            "mlo": mlo, "mhi": mhi,
        })
    return in_maps


def kernel(x, ve, cos, sin, Wq, Wk, Wv, Wg, Wo, window):
    assert int(window) == WINDOW and x.shape == (B, T, C)
    from concourse.bass_utils import run_bass_kernel_spmd

    nc = _get_program()
    in_maps = make_in_maps(x, ve, cos, sin, Wq, Wk, Wv, Wg, Wo)
    res = run_bass_kernel_spmd(nc, in_maps, core_ids=list(range(N_CORES)))
    out = np.zeros((B, T, C), dtype=np.float32)
    for core in range(N_CORES):
        b = core // N_KV
        out[b] += res.results[core]["out"].astype(np.float32)
    return out
